# revision 3
# baseline (speedup 1.0000x reference)
"""Trainium2 Bass kernel for nn_GSA_74045236183284 (histogram_binning), v2.

Sharding: data-parallel over batch B=8 across 8 NeuronCores (1 sample/core),
all params replicated, zero collectives (BatchNorm batch-stats coupling
approximated per-sample; validated far below tolerance).

Structure (per core):
  load x chunks (sync queue, head of line) -> cast to bf16 (vector/gpsimd)
  -> PE transposes (bf16) -> pooled bin sums via accumulating matmuls with
  strided bf16 mask weights; x^2 sums on scalar (Square+accum).
  Attention x3 with transposed-attn trick (softmax denominator folded in as
  a 9th V column), LN rsqrt via Newton iteration on the vector engine (no
  act-table thrash: only exp_and_others + gelu_and_others ever load).
  Closed-form instance/batch-norm stats in column orientation.
  Pass R: scatter matmul + identity-matmul x-add (PSUM accumulate) -> gelu
  straight from PSUM -> g (bf16) + gsum.  SE gates; f2 folded into conv
  weights via diag(f2) matmul; sigmoid via tanh identity.
  Pass F: conv matmul + bias-row matmul -> copy -> store.
"""

import sys

for _p in ("/opt/trn_rl_repo",):
    if _p not in sys.path:
        sys.path.insert(0, _p)

import numpy as np

import concourse.bass as bass
import concourse.bacc as bacc
import concourse.mybir as mybir
import concourse.tile as tile
from concourse.bass_utils import run_bass_kernel_spmd

F32 = mybir.dt.float32
BF16 = mybir.dt.bfloat16
U32 = mybir.dt.uint32
AF = mybir.ActivationFunctionType
ALU = mybir.AluOpType
AX = mybir.AxisListType

B, C, N, K = 8, 128, 16384, 8
NCORES = 8
LOADCH = 1024   # x load chunk (16 chunks)
CH = 1024       # pass R / pass F chunk (16 chunks)
H = C // 2


def build_nc():
    nc = bacc.Bacc("TRN2", target_bir_lowering=False, debug=False,
                   num_devices=NCORES)

    x_d = nc.dram_tensor("x", [C, N], F32, kind="ExternalInput")
    logits_d = nc.dram_tensor("logits", [N], F32, kind="ExternalInput")
    ident_d = nc.dram_tensor("ident", [C, C], F32, kind="ExternalInput")
    w_d = {}
    for nm in ("Wq1", "Wk1", "Wv1", "Wq2", "Wk2", "Wv2", "Wq3", "Wk3", "Wv3",
               "conv0_w"):
        w_d[nm] = nc.dram_tensor(nm, [C, C], F32, kind="ExternalInput")
    fc1w_d = nc.dram_tensor("fc1_w", [H, C], F32, kind="ExternalInput")
    fc2w_d = nc.dram_tensor("fc2_w", [C, H], F32, kind="ExternalInput")
    vecs = {}
    for nm in ("ln_w", "ln_b", "bn_w", "bn_b", "conv0_b", "fc2_b"):
        vecs[nm] = nc.dram_tensor(nm, [C], F32, kind="ExternalInput")
    vecs["fc1_b"] = nc.dram_tensor("fc1_b", [H], F32, kind="ExternalInput")
    out_d = nc.dram_tensor("out", [C, N], BF16, kind="ExternalOutput")

    with tile.TileContext(nc) as tc:
        _body(tc, nc, x_d, logits_d, ident_d, w_d, fc1w_d, fc2w_d, vecs, out_d)

    nc.compile()
    return nc


def _body(tc, nc, x_d, logits_d, ident_d, w_d, fc1w_d, fc2w_d, vecs, out_d):
    from contextlib import ExitStack
    ctx = ExitStack()
    with ctx:
        singles = ctx.enter_context(tc.tile_pool(name="singles", bufs=1))
        xload = ctx.enter_context(tc.tile_pool(name="xload", bufs=6))
        och = ctx.enter_context(tc.tile_pool(name="och", bufs=3))
        scr2 = ctx.enter_context(tc.tile_pool(name="scr2", bufs=2))
        dramp = ctx.enter_context(tc.tile_pool(name="dramp", bufs=1,
                                               space="DRAM"))
        psA = ctx.enter_context(tc.tile_pool(name="psA", bufs=1, space="PSUM"))
        psD_cm = tc.tile_pool(name="psD", bufs=2, space="PSUM")
        psD = psD_cm.__enter__()

        # ---------------- sync queue head: ident + all x chunk loads -------
        ident = singles.tile([C, C], F32)
        nc.sync.dma_start(ident[:], ident_d.ap())
        xts = []
        for ci in range(N // LOADCH):
            xt = xload.tile([C, LOADCH], F32, tag="xt")
            nc.sync.dma_start(xt[:], x_d.ap()[:, ci * LOADCH:(ci + 1) * LOADCH])
            xts.append(xt)

        # ---------------- small loads on gpsimd queue ----------------
        lg = singles.tile([C, C], F32)   # logits as [p, f], n = p*128+f
        nc.gpsimd.dma_start(lg[:], logits_d.ap().rearrange("(p f) -> p f", f=C))
        wsb = {}
        for nm in w_d:
            t = singles.tile([C, C], F32, tag=f"wl_{nm}")
            nc.gpsimd.dma_start(t[:], w_d[nm].ap())
            wsb[nm] = t
        fc1w = singles.tile([H, C], F32)
        nc.gpsimd.dma_start(fc1w[:], fc1w_d.ap())
        fc2w = singles.tile([C, H], F32)
        nc.gpsimd.dma_start(fc2w[:], fc2w_d.ap())
        vrows = singles.tile([7, C], F32)
        nc.vector.memset(vrows[:], 0.0)
        for r, nm in enumerate(("ln_w", "ln_b", "conv0_b", "bn_w", "bn_b")):
            nc.gpsimd.dma_start(vrows[r:r + 1, :], vecs[nm].ap()[None, :])
        nc.gpsimd.dma_start(vrows[5:6, :H], vecs["fc1_b"].ap()[None, :])
        nc.gpsimd.dma_start(vrows[6:7, :], vecs["fc2_b"].ap()[None, :])

        # ---------------- constants ----------------
        identb = singles.tile([C, C], BF16)
        nc.vector.tensor_copy(identb[:], ident[:])
        ones_col = singles.tile([C, 1], F32)
        nc.vector.memset(ones_col[:], 1.0)
        ones_row = singles.tile([1, C], F32)
        nc.vector.memset(ones_row[:], 1.0)
        onesCH_row = singles.tile([1, CH], BF16)
        nc.vector.memset(onesCH_row[:], 1.0)
        ones8 = singles.tile([K, 1], F32)
        nc.vector.memset(ones8[:], 1.0)
        magic = singles.tile([C, K], U32)
        nc.vector.memset(magic[:], 0x5f3759df)
        dummy = singles.tile([1, 1], F32)
        nc.vector.memset(dummy[:], 1.0)

        def rsqrt_newton(v_ap, out_ap, scr_ap, p, w, iters=2):
            # out = 1/sqrt(v) via bit-trick + Newton steps (vector only)
            ou = out_ap.bitcast(U32)
            nc.vector.tensor_scalar(ou, v_ap.bitcast(U32), 1, None,
                                    ALU.logical_shift_right)
            nc.vector.tensor_tensor(ou, magic[:p, :w], ou, ALU.subtract)
            for _ in range(iters):
                nc.vector.tensor_tensor(scr_ap, out_ap, out_ap, ALU.mult)
                nc.vector.tensor_tensor(scr_ap, scr_ap, v_ap, ALU.mult)
                nc.vector.tensor_scalar(scr_ap, scr_ap, -0.5, 1.5,
                                        ALU.mult, ALU.add)
                nc.vector.tensor_tensor(out_ap, out_ap, scr_ap, ALU.mult)

        # ---------------- masks from tanh(logits) (fp32 compares) ---------
        wA = singles.tile([C, C], F32)
        nc.scalar.activation(wA[:], lg[:], AF.Tanh)

        def build_masks(dst, src, nbins, eng):
            for j in range(8):
                lo = -1.0 + 0.25 * j
                eng.tensor_scalar(dst[:, j * C:(j + 1) * C], src[:],
                                  float(lo), None, ALU.is_gt)
            for j in range(7):
                eng.tensor_tensor(dst[:, j * C:(j + 1) * C],
                                  dst[:, j * C:(j + 1) * C],
                                  dst[:, (j + 1) * C:(j + 2) * C],
                                  ALU.subtract)
            neq = scr2.tile([C, C], dst.dtype, tag="neq" + eng.__class__.__name__)
            eng.tensor_scalar(neq[:], src[:], 0.0, None, ALU.not_equal)
            eng.tensor_tensor(dst[:, 3 * C:4 * C], dst[:, 3 * C:4 * C],
                              neq[:], ALU.mult)
            if nbins > 8:
                eng.memset(dst[:, 8 * C:9 * C], 1.0)

        # A-layout masks (bf16): [p, j*128+f], n = p*128+f
        mA = singles.tile([C, 8 * C], BF16)
        build_masks(mA, wA, 8, nc.vector)

        # Mrow [j, n] bf16 via DRAM roundtrip on the scalar HWDGE queue
        mrow_dram = dramp.tile([K, N], BF16)
        for j in range(K):
            nc.scalar.dma_start(mrow_dram[j:j + 1, :].rearrange("o n -> (o n)"),
                                mA[:, j * C:(j + 1) * C])
        Mrow = singles.tile([K, N], BF16)
        nc.scalar.dma_start(Mrow[:], mrow_dram[:])

        # nums: per-bin counts (exact: sums of 0/1)
        numsA = singles.tile([C, K], F32)
        for j in range(K):
            nc.vector.reduce_sum(numsA[:, j:j + 1], mA[:, j * C:(j + 1) * C],
                                 axis=AX.X)
        nums_ps = psD.tile([K, 1], F32, tag="pd")
        nc.tensor.matmul(nums_ps[:], numsA[:], ones_col[:], start=True,
                         stop=True)
        nums_c = singles.tile([K, 1], F32)
        nc.vector.tensor_copy(nums_c[:], nums_ps[:])
        rnums9_c = singles.tile([K + 1, 1], F32)
        nc.vector.memset(rnums9_c[:], 1.0)
        nc.vector.tensor_scalar(rnums9_c[:K], nums_c[:], 1.0, None, ALU.max)
        nc.vector.reciprocal(rnums9_c[:K], rnums9_c[:K])

        # T-layout masks (bf16) from wT: [i, j*128+q] = mask_j(n=q*128+i);
        # block j=8 is ones (sum-of-x row)
        wT_ps = psA.tile([C, C], F32, tag="pa")
        nc.tensor.transpose(wT_ps[:], wA[:], ident[:])
        wT = singles.tile([C, C], F32)
        nc.scalar.copy(wT[:], wT_ps[:])
        mT = singles.tile([C, 9 * C], BF16)
        build_masks(mT, wT, 9, nc.vector)

        # weight transposes (fold 1/temp into WqT)
        temp = float(np.sqrt(np.float32(C)))
        wqkT = []
        wvT = []
        for l in range(3):
            qk = singles.tile([C, 2 * C], F32, tag=f"wqkT{l}")
            ps = psA.tile([C, C], F32, tag="pa")
            nc.tensor.transpose(ps[:], wsb[f"Wq{l+1}"][:], ident[:])
            nc.scalar.activation(qk[:, :C], ps[:], AF.Copy, scale=1.0 / temp)
            ps = psA.tile([C, C], F32, tag="pa")
            nc.tensor.transpose(ps[:], wsb[f"Wk{l+1}"][:], ident[:])
            nc.vector.tensor_copy(qk[:, C:], ps[:])
            wqkT.append(qk)
            vt = singles.tile([C, C], F32, tag=f"wvT{l}")
            ps = psA.tile([C, C], F32, tag="pa")
            nc.tensor.transpose(ps[:], wsb[f"Wv{l+1}"][:], ident[:])
            nc.scalar.copy(vt[:], ps[:])
            wvT.append(vt)
        convwT = singles.tile([C, C], F32)
        ps = psA.tile([C, C], F32, tag="pa")
        nc.tensor.transpose(ps[:], wsb["conv0_w"][:], ident[:])
        nc.vector.tensor_copy(convwT[:], ps[:])
        fc1wT = singles.tile([C, H], F32)
        ps = psA.tile([C, C], F32, tag="pa")
        nc.tensor.transpose(ps[:, :H], fc1w[:], ident[:H, :H])
        nc.vector.tensor_copy(fc1wT[:], ps[:, :H])
        fc2wT = singles.tile([H, C], F32)
        ps = psA.tile([C, C], F32, tag="pa")
        nc.tensor.transpose(ps[:H, :], fc2w[:], ident[:])
        nc.scalar.copy(fc2wT[:], ps[:H, :])
        ps = psA.tile([C, C], F32, tag="pa")
        nc.tensor.transpose(ps[:, :7], vrows[:], ident[:7, :7])
        vcols = singles.tile([C, 7], F32)
        nc.scalar.copy(vcols[:], ps[:, :7])
        lnw_c, lnb_c = vcols[:, 0:1], vcols[:, 1:2]
        convb_c = vcols[:, 2:3]
        bnw_c, bnb_c = vcols[:, 3:4], vcols[:, 4:5]
        fc1b_c = vcols[:H, 5:6]
        halffc2b_c = singles.tile([C, 1], F32)
        nc.vector.tensor_scalar(halffc2b_c[:], vcols[:, 6:7], 0.5, None,
                                ALU.mult)

        # ---------------- x: cast -> PE transpose -> pooled ----------------
        xb = singles.tile([C, N], BF16)      # x in bf16, [c, n]
        xT = singles.tile([C, N], BF16)      # x^T tiles: [i, q*128+c]
        xsq_part = singles.tile([C, N // LOADCH], F32)
        psB_cm = tc.tile_pool(name="psB", bufs=1, space="PSUM")
        psB = psB_cm.__enter__()
        pooled_ps = psB.tile([K + 1, C], F32)
        with tc.tile_pool(name="psT", bufs=4, space="PSUM") as psT:
            for ci in range(N // LOADCH):
                xt = xts[ci]
                xbs = xb[:, ci * LOADCH:(ci + 1) * LOADCH]
                nc.vector.tensor_copy(xbs, xt[:])
                # x^2 channel sums on gpsimd+vector (keep scalar free)
                sqscr = scr2.tile([C, LOADCH], BF16, tag="sqscr")
                nc.gpsimd.tensor_tensor(sqscr[:], xbs, xbs, ALU.mult)
                nc.vector.reduce_sum(xsq_part[:, ci:ci + 1], sqscr[:],
                                     axis=AX.X)
                for grp in range(2):
                    q0 = ci * (LOADCH // C) + grp * 4
                    pt = psT.tile([C, 4 * C], BF16, tag="pt")
                    for g_ in range(4):
                        q = q0 + g_
                        nc.tensor.transpose(pt[:, g_ * C:(g_ + 1) * C],
                                            xb[:, q * C:(q + 1) * C],
                                            identb[:])
                    dst = xT[:, q0 * C:(q0 + 4) * C]
                    nc.scalar.copy(dst, pt[:])
                    for g_ in range(4):
                        q = q0 + g_
                        nc.tensor.matmul(pooled_ps[:], mT[:, q::C],
                                         xT[:, q * C:(q + 1) * C],
                                         start=(q == 0),
                                         stop=(q == N // C - 1))

        # pooledT9: rows 0-7 per-bin sums, row 8 = sum_n x
        pooledT9 = singles.tile([K + 1, C], F32)
        nc.vector.tensor_copy(pooledT9[:], pooled_ps[:])
        pooledT = pooledT9[:K, :]
        psB_cm.__exit__(None, None, None)

        # combo = [featT rows; sumx row] in one op (rnums9 row 8 = 1.0)
        combo = singles.tile([K + 1, C], F32)
        nc.vector.tensor_scalar(combo[:], pooledT9[:], rnums9_c[:], None,
                                ALU.mult)
        cps = psA.tile([C, C], F32, tag="pa")
        nc.tensor.transpose(cps[:, :K + 1], combo[:], ident[:K + 1, :K + 1])
        fea9 = singles.tile([C, K + 1], F32)
        nc.vector.tensor_copy(fea9[:], cps[:, :K + 1])
        fea = fea9[:, :K]
        sumx_col = fea9[:, K:K + 1]

        # ---------------- attention x3 ----------------
        for l in range(3):
            qk_ps = psD.tile([K, 2 * C], F32, tag="pd")
            nc.tensor.matmul(qk_ps[:], fea, wqkT[l][:], start=True, stop=True)
            qkT = singles.tile([K, 2 * C], F32, tag=f"qkT{l}")
            nc.vector.tensor_copy(qkT[:], qk_ps[:])
            v_ps = psD.tile([C, K], F32, tag="pd")
            nc.tensor.matmul(v_ps[:], wvT[l][:], fea, start=True, stop=True)
            v9 = singles.tile([C, K + 1], F32, tag=f"v9{l}")
            nc.vector.tensor_copy(v9[:, :K], v_ps[:])
            nc.vector.memset(v9[:, K:K + 1], 1.0)
            # attn^T directly: atT[d, c] = sum_j k[d,j] q[c,j]
            atT_ps = psA.tile([C, C], F32, tag="pa")
            nc.tensor.matmul(atT_ps[:], qkT[:, C:], qkT[:, :C], start=True,
                             stop=True)
            expT = singles.tile([C, C], F32, tag=f"eT{l}")
            nc.scalar.activation(expT[:], atT_ps[:], AF.Exp)
            # ao9 = expT^T @ v9: cols 0-7 unnormalized out, col 8 = denom
            ao9_ps = psD.tile([C, K + 1], F32, tag="pd")
            nc.tensor.matmul(ao9_ps[:], expT[:], v9[:], start=True, stop=True)
            rse = singles.tile([C, 1], F32, tag=f"rse{l}")
            nc.vector.reciprocal(rse[:], ao9_ps[:, K:K + 1])
            # osb = ao9*rse + fea (residual); stin = [osb | osb^2]
            stin = singles.tile([C, 2 * K], F32, tag=f"stin{l}")
            nc.vector.tensor_scalar(stin[:, :K], ao9_ps[:, :K], rse[:], None,
                                    ALU.mult)
            nc.vector.tensor_tensor(stin[:, :K], stin[:, :K], fea, ALU.add)
            nc.vector.tensor_tensor(stin[:, K:], stin[:, :K], stin[:, :K],
                                    ALU.mult)
            # LN over c (partition dim): st = ones^T @ stin -> [1, 16]
            st_ps = psD.tile([1, 2 * K], F32, tag="pd")
            nc.tensor.matmul(st_ps[:], ones_col[:], stin[:], start=True,
                             stop=True)
            mr = singles.tile([1, 2 * K], F32, tag=f"mr{l}")
            nc.vector.tensor_scalar(mr[:], st_ps[:], 1.0 / C, None, ALU.mult)
            # var = E[x^2] - mu^2 ; rs = 1/sqrt(var+eps) via newton (vector)
            vs8 = singles.tile([1, 2 * K], F32, tag=f"vs8{l}")
            nc.vector.tensor_tensor(vs8[:, :K], mr[:, :K], mr[:, :K], ALU.mult)
            nc.vector.tensor_tensor(vs8[:, :K], mr[:, K:], vs8[:, :K],
                                    ALU.subtract)
            nc.vector.tensor_scalar(vs8[:, :K], vs8[:, :K], 1e-6, None,
                                    ALU.add)
            rsqrt_newton(vs8[:, :K], mr[:, K:], vs8[:, K:], 1, K, iters=1)
            # broadcast [mu | rs] down partitions
            bc_ps = psD.tile([C, 2 * K], F32, tag="pd")
            nc.tensor.matmul(bc_ps[:], ones_row[:], mr[:], start=True,
                             stop=True)
            fea2 = singles.tile([C, K], F32, tag=f"fea{l+1}")
            nc.vector.tensor_tensor(fea2[:], stin[:, :K], bc_ps[:, :K],
                                    ALU.subtract)
            nc.vector.tensor_tensor(fea2[:], fea2[:], bc_ps[:, K:], ALU.mult)
            nc.vector.tensor_scalar(fea2[:], fea2[:], lnw_c, lnb_c,
                                    ALU.mult, ALU.add)
            fea = fea2[:]

        # ---------------- closed-form instance/batch-norm stats -----------
        ftp = psA.tile([C, C], F32, tag="pa")
        nc.tensor.transpose(ftp[:K, :], fea, ident[:])
        featb = singles.tile([K, C], BF16)
        nc.vector.tensor_copy(featb[:], ftp[:K, :])
        featr = singles.tile([K, C], F32)
        nc.vector.tensor_copy(featr[:], featb[:])
        fsq = singles.tile([K, C], F32)
        nc.vector.tensor_tensor(fsq[:], featr[:], featr[:], ALU.mult)
        prod = singles.tile([K, C], F32)
        nc.vector.tensor_tensor(prod[:], featr[:], pooledT, ALU.mult)
        r1_ps = psD.tile([C, 1], F32, tag="pd")
        nc.tensor.matmul(r1_ps[:], featr[:], nums_c[:], start=True, stop=True)
        r2_ps = psD.tile([C, 1], F32, tag="pd")
        nc.tensor.matmul(r2_ps[:], fsq[:], nums_c[:], start=True, stop=True)
        r3_ps = psA.tile([C, 1], F32, tag="pa")
        nc.tensor.matmul(r3_ps[:], prod[:], ones8[:], start=True, stop=True)

        xsq_col = singles.tile([C, 1], F32)
        nc.vector.reduce_sum(xsq_col[:], xsq_part[:], axis=AX.X)
        stats = singles.tile([C, 8], F32)
        mu = stats[:, 0:1]
        var = stats[:, 1:2]
        rsb = stats[:, 2:3]
        s_col = stats[:, 3:4]
        b_col = stats[:, 4:5]
        tmp = stats[:, 5:6]
        nc.vector.tensor_tensor(tmp[:], sumx_col, r1_ps[:], ALU.add)
        nc.vector.tensor_scalar(mu[:], tmp[:], 1.0 / N, None, ALU.mult)
        nc.vector.tensor_scalar(tmp[:], r3_ps[:], 2.0, None, ALU.mult)
        nc.vector.tensor_tensor(tmp[:], tmp[:], r2_ps[:], ALU.add)
        nc.vector.tensor_tensor(tmp[:], tmp[:], xsq_col[:], ALU.add)
        nc.vector.tensor_scalar(tmp[:], tmp[:], 1.0 / N, None, ALU.mult)
        nc.vector.tensor_tensor(var[:], mu[:], mu[:], ALU.mult)
        nc.vector.tensor_tensor(var[:], tmp[:], var[:], ALU.subtract)
        # rs_i = 1/sqrt(var+1e-5); vb = var/(var+1e-5); rs_b = 1/sqrt(vb+1e-5)
        rsi = stats[:, 6:7]
        inv = stats[:, 7:8]
        scr2c = singles.tile([C, 2], F32, tag="nsc")
        nc.vector.tensor_scalar(inv[:], var[:], 1e-5, None, ALU.add)
        rsqrt_newton(inv[:], rsi[:], scr2c[:, 0:1], C, 1)
        nc.vector.reciprocal(inv[:], inv[:])
        nc.vector.tensor_tensor(rsb[:], var[:], inv[:], ALU.mult)
        nc.vector.tensor_scalar(rsb[:], rsb[:], 1e-5, None, ALU.add)
        rsqrt_newton(rsb[:], scr2c[:, 1:2], scr2c[:, 0:1], C, 1)
        nc.vector.tensor_copy(rsb[:], scr2c[:, 1:2])
        # dummy gelu: pull the gelu-table load off the critical path
        nc.scalar.activation(dummy[:], dummy[:], AF.Gelu)
        nc.vector.tensor_tensor(s_col[:], rsi[:], rsb[:], ALU.mult)
        nc.vector.tensor_tensor(s_col[:], s_col[:], bnw_c, ALU.mult)
        nc.vector.tensor_tensor(b_col[:], mu[:], s_col[:], ALU.mult)
        nc.vector.tensor_tensor(b_col[:], bnb_c, b_col[:], ALU.subtract)

        psD_cm.__exit__(None, None, None)

        # ---------------- pass R + gates + pass F ----------------
        g = singles.tile([C, N], BF16)
        gsum_part = singles.tile([C, N // CH], F32)
        with tc.tile_pool(name="psC", bufs=3, space="PSUM") as psC:
            for r in range(N // CH):
                off = r * CH
                sc_ps = psC.tile([C, CH], F32, tag="pc")
                for h in range(CH // 512):
                    o2 = off + h * 512
                    sl = sc_ps[:, h * 512:(h + 1) * 512]
                    nc.tensor.matmul(sl, featb[:], Mrow[:, o2:o2 + 512],
                                     start=True, stop=False)
                    nc.tensor.matmul(sl, identb[:], xb[:, o2:o2 + 512],
                                     start=False, stop=True)
                nc.scalar.activation(g[:, off:off + CH], sc_ps[:], AF.Gelu,
                                     bias=b_col, scale=s_col,
                                     accum_out=gsum_part[:, r:r + 1])

            # SE gates
            gsum_col = singles.tile([C, 1], F32)
            nc.vector.reduce_sum(gsum_col[:], gsum_part[:], axis=AX.X)
            sq_ps = psA.tile([C, 1], F32, tag="pa")
            nc.tensor.matmul(sq_ps[:], convwT[:], gsum_col[:], start=True,
                             stop=True)
            sq = singles.tile([C, 1], F32)
            nc.vector.tensor_scalar(sq[:], sq_ps[:], 1.0 / N, convb_c,
                                    ALU.mult, ALU.add)
            f1_ps = psA.tile([H, 1], F32, tag="pa")
            nc.tensor.matmul(f1_ps[:], fc1wT[:], sq[:], start=True, stop=True)
            f1 = singles.tile([H, 1], F32)
            nc.scalar.activation(f1[:], f1_ps[:], AF.Gelu, bias=fc1b_c)
            f2_ps = psA.tile([C, 1], F32, tag="pa")
            nc.tensor.matmul(f2_ps[:], fc2wT[:], f1[:], start=True, stop=True)
            # sigmoid(z) = 0.5*tanh(z/2) + 0.5 (tanh is in the gelu table)
            f2 = singles.tile([C, 1], F32)
            nc.scalar.activation(f2[:], f2_ps[:], AF.Tanh, scale=0.5,
                                 bias=halffc2b_c)
            nc.vector.tensor_scalar(f2[:], f2[:], 0.5, 0.5, ALU.mult, ALU.add)
            fb = singles.tile([C, 1], F32)     # f2 * conv0_b
            nc.vector.tensor_tensor(fb[:], f2[:], convb_c, ALU.mult)
            # W''[c, o] = convwT[c, o] * f2[o]  via  conv0_w @ diag(f2)
            diag = singles.tile([C, C], F32)
            nc.vector.tensor_scalar(diag[:], ident[:], f2[:], None, ALU.mult)
            wpp_ps = psA.tile([C, C], F32, tag="pa")
            nc.tensor.matmul(wpp_ps[:], wsb["conv0_w"][:], diag[:], start=True,
                             stop=True)
            wpp = singles.tile([C, C], BF16)
            nc.vector.tensor_copy(wpp[:], wpp_ps[:])

            # pass F: conv + bias-row matmul -> copy -> store
            for r in range(N // CH):
                off = r * CH
                cv_ps = psC.tile([C, CH], F32, tag="pc")
                for h in range(CH // 512):
                    o2 = off + h * 512
                    nc.tensor.matmul(cv_ps[:, h * 512:(h + 1) * 512], wpp[:],
                                     g[:, o2:o2 + 512], start=True, stop=True)
                ot = och.tile([C, CH], BF16, tag="ot")
                if r % 2 == 0:
                    nc.vector.tensor_scalar(ot[:], cv_ps[:], fb[:], None,
                                            ALU.add)
                else:
                    nc.scalar.activation(ot[:], cv_ps[:], AF.Identity,
                                         bias=fb[:])
                nc.sync.dma_start(out_d.ap()[:, off:off + CH], ot[:])


_NC_CACHE = {}


def _get_nc():
    if "nc" not in _NC_CACHE:
        _NC_CACHE["nc"] = build_nc()
    return _NC_CACHE["nc"]


def kernel(**inputs):
    x = np.ascontiguousarray(np.asarray(inputs["x"], dtype=np.float32))
    logits = np.ascontiguousarray(np.asarray(inputs["logits"],
                                             dtype=np.float32))
    assert x.shape == (B, C, N, 1) and logits.shape == (B, N)
    ident = np.eye(C, dtype=np.float32)
    shared = {"ident": ident}
    for nm in ("Wq1", "Wk1", "Wv1", "Wq2", "Wk2", "Wv2", "Wq3", "Wk3", "Wv3",
               "conv0_w", "fc1_w", "fc2_w", "ln_w", "ln_b", "bn_w", "bn_b",
               "conv0_b", "fc1_b", "fc2_b"):
        shared[nm] = np.ascontiguousarray(np.asarray(inputs[nm],
                                                     dtype=np.float32))
    in_maps = []
    for i in range(NCORES):
        m = dict(shared)
        m["x"] = np.ascontiguousarray(x[i, :, :, 0])
        m["logits"] = np.ascontiguousarray(logits[i])
        in_maps.append(m)

    nc = _get_nc()
    res = run_bass_kernel_spmd(nc, in_maps, list(range(NCORES))).results
    out = np.stack([res[i]["out"] for i in range(NCORES)], axis=0)
    return out[..., None].astype(np.float32)


# revision 4
# speedup vs baseline: 1.0081x; 1.0081x over previous
"""Trainium2 Bass kernel for nn_GSA_74045236183284 (histogram_binning), v2.

Sharding: data-parallel over batch B=8 across 8 NeuronCores (1 sample/core),
all params replicated, zero collectives (BatchNorm batch-stats coupling
approximated per-sample; validated far below tolerance).

Structure (per core):
  load x chunks (sync queue, head of line) -> cast to bf16 (vector/gpsimd)
  -> PE transposes (bf16) -> pooled bin sums via accumulating matmuls with
  strided bf16 mask weights; x^2 sums on scalar (Square+accum).
  Attention x3 with transposed-attn trick (softmax denominator folded in as
  a 9th V column), LN rsqrt via Newton iteration on the vector engine (no
  act-table thrash: only exp_and_others + gelu_and_others ever load).
  Closed-form instance/batch-norm stats in column orientation.
  Pass R: scatter matmul + identity-matmul x-add (PSUM accumulate) -> gelu
  straight from PSUM -> g (bf16) + gsum.  SE gates; f2 folded into conv
  weights via diag(f2) matmul; sigmoid via tanh identity.
  Pass F: conv matmul + bias-row matmul -> copy -> store.
"""

import sys

for _p in ("/opt/trn_rl_repo",):
    if _p not in sys.path:
        sys.path.insert(0, _p)

import numpy as np

import concourse.bass as bass
import concourse.bacc as bacc
import concourse.mybir as mybir
import concourse.tile as tile
from concourse.bass_utils import run_bass_kernel_spmd

F32 = mybir.dt.float32
BF16 = mybir.dt.bfloat16
U32 = mybir.dt.uint32
AF = mybir.ActivationFunctionType
ALU = mybir.AluOpType
AX = mybir.AxisListType

B, C, N, K = 8, 128, 16384, 8
NCORES = 8
LOADCH = 1024   # x load chunk (16 chunks)
CH = 1024       # pass R / pass F chunk (16 chunks)
H = C // 2


def build_nc():
    nc = bacc.Bacc("TRN2", target_bir_lowering=False, debug=False,
                   num_devices=NCORES)

    x_d = nc.dram_tensor("x", [C, N], F32, kind="ExternalInput")
    logits_d = nc.dram_tensor("logits", [N], F32, kind="ExternalInput")
    ident_d = nc.dram_tensor("ident", [C, C], F32, kind="ExternalInput")
    w_d = {}
    for nm in ("Wq1", "Wk1", "Wv1", "Wq2", "Wk2", "Wv2", "Wq3", "Wk3", "Wv3",
               "conv0_w"):
        w_d[nm] = nc.dram_tensor(nm, [C, C], F32, kind="ExternalInput")
    fc1w_d = nc.dram_tensor("fc1_w", [H, C], F32, kind="ExternalInput")
    fc2w_d = nc.dram_tensor("fc2_w", [C, H], F32, kind="ExternalInput")
    vecs = {}
    for nm in ("ln_w", "ln_b", "bn_w", "bn_b", "conv0_b", "fc2_b"):
        vecs[nm] = nc.dram_tensor(nm, [C], F32, kind="ExternalInput")
    vecs["fc1_b"] = nc.dram_tensor("fc1_b", [H], F32, kind="ExternalInput")
    out_d = nc.dram_tensor("out", [C, N], BF16, kind="ExternalOutput")

    with tile.TileContext(nc) as tc:
        _body(tc, nc, x_d, logits_d, ident_d, w_d, fc1w_d, fc2w_d, vecs, out_d)

    nc.compile()
    return nc


def _body(tc, nc, x_d, logits_d, ident_d, w_d, fc1w_d, fc2w_d, vecs, out_d):
    from contextlib import ExitStack
    ctx = ExitStack()
    with ctx:
        singles = ctx.enter_context(tc.tile_pool(name="singles", bufs=1))
        xload = ctx.enter_context(tc.tile_pool(name="xload", bufs=6))
        och = ctx.enter_context(tc.tile_pool(name="och", bufs=3))
        scr2 = ctx.enter_context(tc.tile_pool(name="scr2", bufs=2))
        dramp = ctx.enter_context(tc.tile_pool(name="dramp", bufs=1,
                                               space="DRAM"))
        psA = ctx.enter_context(tc.tile_pool(name="psA", bufs=1, space="PSUM"))
        psD_cm = tc.tile_pool(name="psD", bufs=2, space="PSUM")
        psD = psD_cm.__enter__()

        # ---------------- sync queue head: ident + all x chunk loads -------
        ident = singles.tile([C, C], F32)
        nc.sync.dma_start(ident[:], ident_d.ap())
        xts = []
        for ci in range(N // LOADCH):
            xt = xload.tile([C, LOADCH], F32, tag="xt")
            nc.sync.dma_start(xt[:], x_d.ap()[:, ci * LOADCH:(ci + 1) * LOADCH])
            xts.append(xt)

        # ---------------- small loads on gpsimd queue ----------------
        lg = singles.tile([C, C], F32)   # logits as [p, f], n = p*128+f
        nc.gpsimd.dma_start(lg[:], logits_d.ap().rearrange("(p f) -> p f", f=C))
        wsb = {}
        for nm in w_d:
            t = singles.tile([C, C], F32, tag=f"wl_{nm}")
            nc.gpsimd.dma_start(t[:], w_d[nm].ap())
            wsb[nm] = t
        fc1w = singles.tile([H, C], F32)
        nc.gpsimd.dma_start(fc1w[:], fc1w_d.ap())
        fc2w = singles.tile([C, H], F32)
        nc.gpsimd.dma_start(fc2w[:], fc2w_d.ap())
        vrows = singles.tile([7, C], F32)
        nc.vector.memset(vrows[:], 0.0)
        for r, nm in enumerate(("ln_w", "ln_b", "conv0_b", "bn_w", "bn_b")):
            nc.gpsimd.dma_start(vrows[r:r + 1, :], vecs[nm].ap()[None, :])
        nc.gpsimd.dma_start(vrows[5:6, :H], vecs["fc1_b"].ap()[None, :])
        nc.gpsimd.dma_start(vrows[6:7, :], vecs["fc2_b"].ap()[None, :])

        # ---------------- constants ----------------
        identb = singles.tile([C, C], BF16)
        nc.vector.tensor_copy(identb[:], ident[:])
        ones_col = singles.tile([C, 1], F32)
        nc.vector.memset(ones_col[:], 1.0)
        ones_row = singles.tile([1, C], F32)
        nc.vector.memset(ones_row[:], 1.0)
        onesCH_row = singles.tile([1, CH], BF16)
        nc.vector.memset(onesCH_row[:], 1.0)
        ones8 = singles.tile([K, 1], F32)
        nc.vector.memset(ones8[:], 1.0)
        magic = singles.tile([C, K], U32)
        nc.vector.memset(magic[:], 0x5f3759df)
        dummy = singles.tile([1, 1], F32)
        nc.vector.memset(dummy[:], 1.0)

        def rsqrt_newton(v_ap, out_ap, scr_ap, p, w, iters=2):
            # out = 1/sqrt(v) via bit-trick + Newton steps (vector only)
            ou = out_ap.bitcast(U32)
            nc.vector.tensor_scalar(ou, v_ap.bitcast(U32), 1, None,
                                    ALU.logical_shift_right)
            nc.vector.tensor_tensor(ou, magic[:p, :w], ou, ALU.subtract)
            for _ in range(iters):
                nc.vector.tensor_tensor(scr_ap, out_ap, out_ap, ALU.mult)
                nc.vector.tensor_tensor(scr_ap, scr_ap, v_ap, ALU.mult)
                nc.vector.tensor_scalar(scr_ap, scr_ap, -0.5, 1.5,
                                        ALU.mult, ALU.add)
                nc.vector.tensor_tensor(out_ap, out_ap, scr_ap, ALU.mult)

        # ---------------- masks from tanh(logits) (fp32 compares) ---------
        wA = singles.tile([C, C], F32)
        nc.scalar.activation(wA[:], lg[:], AF.Tanh)

        def build_masks(dst, src, nbins, eng):
            for j in range(8):
                lo = -1.0 + 0.25 * j
                eng.tensor_scalar(dst[:, j * C:(j + 1) * C], src[:],
                                  float(lo), None, ALU.is_gt)
            for j in range(7):
                eng.tensor_tensor(dst[:, j * C:(j + 1) * C],
                                  dst[:, j * C:(j + 1) * C],
                                  dst[:, (j + 1) * C:(j + 2) * C],
                                  ALU.subtract)
            neq = scr2.tile([C, C], dst.dtype, tag="neq" + eng.__class__.__name__)
            eng.tensor_scalar(neq[:], src[:], 0.0, None, ALU.not_equal)
            eng.tensor_tensor(dst[:, 3 * C:4 * C], dst[:, 3 * C:4 * C],
                              neq[:], ALU.mult)
            if nbins > 8:
                eng.memset(dst[:, 8 * C:9 * C], 1.0)

        # A-layout masks (bf16): [p, j*128+f], n = p*128+f
        mA = singles.tile([C, 8 * C], BF16)
        build_masks(mA, wA, 8, nc.vector)

        # Mrow [j, n] bf16 via DRAM roundtrip on the scalar HWDGE queue
        mrow_dram = dramp.tile([K, N], BF16)
        for j in range(K):
            nc.scalar.dma_start(mrow_dram[j:j + 1, :].rearrange("o n -> (o n)"),
                                mA[:, j * C:(j + 1) * C])
        Mrow = singles.tile([K, N], BF16)
        nc.scalar.dma_start(Mrow[:], mrow_dram[:])

        # nums: per-bin counts (exact: sums of 0/1)
        numsA = singles.tile([C, K], F32)
        for j in range(K):
            nc.vector.reduce_sum(numsA[:, j:j + 1], mA[:, j * C:(j + 1) * C],
                                 axis=AX.X)
        nums_ps = psD.tile([K, 1], F32, tag="pd")
        nc.tensor.matmul(nums_ps[:], numsA[:], ones_col[:], start=True,
                         stop=True)
        nums_c = singles.tile([K, 1], F32)
        nc.vector.tensor_copy(nums_c[:], nums_ps[:])
        rnums9_c = singles.tile([K + 1, 1], F32)
        nc.vector.memset(rnums9_c[:], 1.0)
        nc.vector.tensor_scalar(rnums9_c[:K], nums_c[:], 1.0, None, ALU.max)
        nc.vector.reciprocal(rnums9_c[:K], rnums9_c[:K])

        # T-layout masks (bf16) from wT: [i, j*128+q] = mask_j(n=q*128+i);
        # block j=8 is ones (sum-of-x row)
        wT_ps = psA.tile([C, C], F32, tag="pa")
        nc.tensor.transpose(wT_ps[:], wA[:], ident[:])
        wT = singles.tile([C, C], F32)
        nc.scalar.copy(wT[:], wT_ps[:])
        mT = singles.tile([C, 9 * C], BF16)
        build_masks(mT, wT, 9, nc.vector)

        # weight transposes (fold 1/temp into WqT)
        temp = float(np.sqrt(np.float32(C)))
        wqkT = []
        wvT = []
        for l in range(3):
            qk = singles.tile([C, 2 * C], F32, tag=f"wqkT{l}")
            ps = psA.tile([C, C], F32, tag="pa")
            nc.tensor.transpose(ps[:], wsb[f"Wq{l+1}"][:], ident[:])
            nc.scalar.activation(qk[:, :C], ps[:], AF.Copy, scale=1.0 / temp)
            ps = psA.tile([C, C], F32, tag="pa")
            nc.tensor.transpose(ps[:], wsb[f"Wk{l+1}"][:], ident[:])
            nc.scalar.copy(qk[:, C:], ps[:])
            wqkT.append(qk)
            vt = singles.tile([C, C], F32, tag=f"wvT{l}")
            ps = psA.tile([C, C], F32, tag="pa")
            nc.tensor.transpose(ps[:], wsb[f"Wv{l+1}"][:], ident[:])
            nc.scalar.copy(vt[:], ps[:])
            wvT.append(vt)
        convwT = singles.tile([C, C], F32)
        ps = psA.tile([C, C], F32, tag="pa")
        nc.tensor.transpose(ps[:], wsb["conv0_w"][:], ident[:])
        nc.scalar.copy(convwT[:], ps[:])
        fc1wT = singles.tile([C, H], F32)
        ps = psA.tile([C, C], F32, tag="pa")
        nc.tensor.transpose(ps[:, :H], fc1w[:], ident[:H, :H])
        nc.scalar.copy(fc1wT[:], ps[:, :H])
        fc2wT = singles.tile([H, C], F32)
        ps = psA.tile([C, C], F32, tag="pa")
        nc.tensor.transpose(ps[:H, :], fc2w[:], ident[:])
        nc.scalar.copy(fc2wT[:], ps[:H, :])
        ps = psA.tile([C, C], F32, tag="pa")
        nc.tensor.transpose(ps[:, :7], vrows[:], ident[:7, :7])
        vcols = singles.tile([C, 7], F32)
        nc.scalar.copy(vcols[:], ps[:, :7])
        lnw_c, lnb_c = vcols[:, 0:1], vcols[:, 1:2]
        convb_c = vcols[:, 2:3]
        bnw_c, bnb_c = vcols[:, 3:4], vcols[:, 4:5]
        fc1b_c = vcols[:H, 5:6]
        halffc2b_c = singles.tile([C, 1], F32)
        nc.vector.tensor_scalar(halffc2b_c[:], vcols[:, 6:7], 0.5, None,
                                ALU.mult)

        # ---------------- x: cast -> PE transpose -> pooled ----------------
        xb = singles.tile([C, N], BF16)      # x in bf16, [c, n]
        xT = singles.tile([C, N], BF16)      # x^T tiles: [i, q*128+c]
        xsq_part = singles.tile([C, N // LOADCH], F32)
        psB_cm = tc.tile_pool(name="psB", bufs=1, space="PSUM")
        psB = psB_cm.__enter__()
        pooled_ps = psB.tile([K + 1, C], F32)
        with tc.tile_pool(name="psT", bufs=4, space="PSUM") as psT:
            for ci in range(N // LOADCH):
                xt = xts[ci]
                xbs = xb[:, ci * LOADCH:(ci + 1) * LOADCH]
                nc.vector.tensor_copy(xbs, xt[:])

                for grp in range(2):
                    q0 = ci * (LOADCH // C) + grp * 4
                    pt = psT.tile([C, 4 * C], BF16, tag="pt")
                    for g_ in range(4):
                        q = q0 + g_
                        nc.tensor.transpose(pt[:, g_ * C:(g_ + 1) * C],
                                            xb[:, q * C:(q + 1) * C],
                                            identb[:])
                    dst = xT[:, q0 * C:(q0 + 4) * C]
                    nc.scalar.copy(dst, pt[:])
                    for g_ in range(4):
                        q = q0 + g_
                        nc.tensor.matmul(pooled_ps[:], mT[:, q::C],
                                         xT[:, q * C:(q + 1) * C],
                                         start=(q == 0),
                                         stop=(q == N // C - 1))

        # pooledT9: rows 0-7 per-bin sums, row 8 = sum_n x
        pooledT9 = singles.tile([K + 1, C], F32)
        nc.vector.tensor_copy(pooledT9[:], pooled_ps[:])
        pooledT = pooledT9[:K, :]
        psB_cm.__exit__(None, None, None)

        # combo = [featT rows; sumx row] in one op (rnums9 row 8 = 1.0)
        combo = singles.tile([K + 1, C], F32)
        nc.vector.tensor_scalar(combo[:], pooledT9[:], rnums9_c[:], None,
                                ALU.mult)
        cps = psA.tile([C, C], F32, tag="pa")
        nc.tensor.transpose(cps[:, :K + 1], combo[:], ident[:K + 1, :K + 1])
        fea9 = singles.tile([C, K + 1], F32)
        nc.vector.tensor_copy(fea9[:], cps[:, :K + 1])
        fea = fea9[:, :K]
        sumx_col = fea9[:, K:K + 1]

        # x^2 channel sums on scalar (scheduler overlaps these with the
        # pooled tail / attention window)
        for ci in range(N // LOADCH):
            sqscr = scr2.tile([C, LOADCH], BF16, tag="sqscr")
            nc.scalar.activation(sqscr[:], xb[:, ci * LOADCH:(ci + 1) * LOADCH],
                                 AF.Square, accum_out=xsq_part[:, ci:ci + 1])

        # ---------------- attention x3 ----------------
        for l in range(3):
            qk_ps = psD.tile([K, 2 * C], F32, tag="pd")
            nc.tensor.matmul(qk_ps[:], fea, wqkT[l][:], start=True, stop=True)
            qkT = singles.tile([K, 2 * C], F32, tag=f"qkT{l}")
            nc.vector.tensor_copy(qkT[:], qk_ps[:])
            v_ps = psD.tile([C, K], F32, tag="pd")
            nc.tensor.matmul(v_ps[:], wvT[l][:], fea, start=True, stop=True)
            v9 = singles.tile([C, K + 1], F32, tag=f"v9{l}")
            nc.vector.tensor_copy(v9[:, :K], v_ps[:])
            nc.vector.memset(v9[:, K:K + 1], 1.0)
            # attn^T directly: atT[d, c] = sum_j k[d,j] q[c,j]
            atT_ps = psA.tile([C, C], F32, tag="pa")
            nc.tensor.matmul(atT_ps[:], qkT[:, C:], qkT[:, :C], start=True,
                             stop=True)
            expT = singles.tile([C, C], F32, tag=f"eT{l}")
            nc.scalar.activation(expT[:], atT_ps[:], AF.Exp)
            # ao9 = expT^T @ v9: cols 0-7 unnormalized out, col 8 = denom
            ao9_ps = psD.tile([C, K + 1], F32, tag="pd")
            nc.tensor.matmul(ao9_ps[:], expT[:], v9[:], start=True, stop=True)
            rse = singles.tile([C, 1], F32, tag=f"rse{l}")
            nc.vector.reciprocal(rse[:], ao9_ps[:, K:K + 1])
            # osb = ao9*rse + fea (residual); stin = [osb | osb^2]
            stin = singles.tile([C, 2 * K], F32, tag=f"stin{l}")
            nc.vector.tensor_scalar(stin[:, :K], ao9_ps[:, :K], rse[:], None,
                                    ALU.mult)
            nc.vector.tensor_tensor(stin[:, :K], stin[:, :K], fea, ALU.add)
            nc.vector.tensor_tensor(stin[:, K:], stin[:, :K], stin[:, :K],
                                    ALU.mult)
            # LN over c (partition dim): st = ones^T @ stin -> [1, 16]
            st_ps = psD.tile([1, 2 * K], F32, tag="pd")
            nc.tensor.matmul(st_ps[:], ones_col[:], stin[:], start=True,
                             stop=True)
            mr = singles.tile([1, 2 * K], F32, tag=f"mr{l}")
            nc.vector.tensor_scalar(mr[:], st_ps[:], 1.0 / C, None, ALU.mult)
            # var = E[x^2] - mu^2 ; rs = 1/sqrt(var+eps) via newton (vector)
            vs8 = singles.tile([1, 2 * K], F32, tag=f"vs8{l}")
            nc.vector.tensor_tensor(vs8[:, :K], mr[:, :K], mr[:, :K], ALU.mult)
            nc.vector.tensor_tensor(vs8[:, :K], mr[:, K:], vs8[:, :K],
                                    ALU.subtract)
            nc.vector.tensor_scalar(vs8[:, :K], vs8[:, :K], 1e-6, None,
                                    ALU.add)
            rsqrt_newton(vs8[:, :K], mr[:, K:], vs8[:, K:], 1, K, iters=1)
            # broadcast [mu | rs] down partitions
            bc_ps = psD.tile([C, 2 * K], F32, tag="pd")
            nc.tensor.matmul(bc_ps[:], ones_row[:], mr[:], start=True,
                             stop=True)
            fea2 = singles.tile([C, K], F32, tag=f"fea{l+1}")
            nc.vector.tensor_tensor(fea2[:], stin[:, :K], bc_ps[:, :K],
                                    ALU.subtract)
            nc.vector.tensor_tensor(fea2[:], fea2[:], bc_ps[:, K:], ALU.mult)
            nc.vector.tensor_scalar(fea2[:], fea2[:], lnw_c, lnb_c,
                                    ALU.mult, ALU.add)
            fea = fea2[:]

        # ---------------- closed-form instance/batch-norm stats -----------
        ftp = psA.tile([C, C], F32, tag="pa")
        nc.tensor.transpose(ftp[:K, :], fea, ident[:])
        featb = singles.tile([K, C], BF16)
        nc.vector.tensor_copy(featb[:], ftp[:K, :])
        featr = singles.tile([K, C], F32)
        nc.vector.tensor_copy(featr[:], featb[:])
        fsq = singles.tile([K, C], F32)
        nc.vector.tensor_tensor(fsq[:], featr[:], featr[:], ALU.mult)
        prod = singles.tile([K, C], F32)
        nc.vector.tensor_tensor(prod[:], featr[:], pooledT, ALU.mult)
        r1_ps = psD.tile([C, 1], F32, tag="pd")
        nc.tensor.matmul(r1_ps[:], featr[:], nums_c[:], start=True, stop=True)
        r2_ps = psD.tile([C, 1], F32, tag="pd")
        nc.tensor.matmul(r2_ps[:], fsq[:], nums_c[:], start=True, stop=True)
        r3_ps = psA.tile([C, 1], F32, tag="pa")
        nc.tensor.matmul(r3_ps[:], prod[:], ones8[:], start=True, stop=True)

        xsq_col = singles.tile([C, 1], F32)
        nc.vector.reduce_sum(xsq_col[:], xsq_part[:], axis=AX.X)
        stats = singles.tile([C, 8], F32)
        mu = stats[:, 0:1]
        var = stats[:, 1:2]
        rsb = stats[:, 2:3]
        s_col = stats[:, 3:4]
        b_col = stats[:, 4:5]
        tmp = stats[:, 5:6]
        nc.vector.tensor_tensor(tmp[:], sumx_col, r1_ps[:], ALU.add)
        nc.vector.tensor_scalar(mu[:], tmp[:], 1.0 / N, None, ALU.mult)
        nc.vector.tensor_scalar(tmp[:], r3_ps[:], 2.0, None, ALU.mult)
        nc.vector.tensor_tensor(tmp[:], tmp[:], r2_ps[:], ALU.add)
        nc.vector.tensor_tensor(tmp[:], tmp[:], xsq_col[:], ALU.add)
        nc.vector.tensor_scalar(tmp[:], tmp[:], 1.0 / N, None, ALU.mult)
        nc.vector.tensor_tensor(var[:], mu[:], mu[:], ALU.mult)
        nc.vector.tensor_tensor(var[:], tmp[:], var[:], ALU.subtract)
        # rs_i = 1/sqrt(var+1e-5); vb = var/(var+1e-5); rs_b = 1/sqrt(vb+1e-5)
        rsi = stats[:, 6:7]
        inv = stats[:, 7:8]
        scr2c = singles.tile([C, 2], F32, tag="nsc")
        nc.vector.tensor_scalar(inv[:], var[:], 1e-5, None, ALU.add)
        rsqrt_newton(inv[:], rsi[:], scr2c[:, 0:1], C, 1)
        nc.vector.reciprocal(inv[:], inv[:])
        nc.vector.tensor_tensor(rsb[:], var[:], inv[:], ALU.mult)
        nc.vector.tensor_scalar(rsb[:], rsb[:], 1e-5, None, ALU.add)
        rsqrt_newton(rsb[:], scr2c[:, 1:2], scr2c[:, 0:1], C, 1)
        nc.vector.tensor_copy(rsb[:], scr2c[:, 1:2])
        # dummy gelu: pull the gelu-table load off the critical path
        nc.scalar.activation(dummy[:], dummy[:], AF.Gelu)
        nc.vector.tensor_tensor(s_col[:], rsi[:], rsb[:], ALU.mult)
        nc.vector.tensor_tensor(s_col[:], s_col[:], bnw_c, ALU.mult)
        nc.vector.tensor_tensor(b_col[:], mu[:], s_col[:], ALU.mult)
        nc.vector.tensor_tensor(b_col[:], bnb_c, b_col[:], ALU.subtract)

        psD_cm.__exit__(None, None, None)

        # ---------------- pass R + gates + pass F ----------------
        g = singles.tile([C, N], BF16)
        gsum_part = singles.tile([C, N // CH], F32)
        with tc.tile_pool(name="psC", bufs=3, space="PSUM") as psC:
            for r in range(N // CH):
                off = r * CH
                sc_ps = psC.tile([C, CH], F32, tag="pc")
                for h in range(CH // 512):
                    o2 = off + h * 512
                    sl = sc_ps[:, h * 512:(h + 1) * 512]
                    nc.tensor.matmul(sl, featb[:], Mrow[:, o2:o2 + 512],
                                     start=True, stop=False)
                    nc.tensor.matmul(sl, identb[:], xb[:, o2:o2 + 512],
                                     start=False, stop=True)
                nc.scalar.activation(g[:, off:off + CH], sc_ps[:], AF.Gelu,
                                     bias=b_col, scale=s_col,
                                     accum_out=gsum_part[:, r:r + 1])

            # SE gates
            gsum_col = singles.tile([C, 1], F32)
            nc.vector.reduce_sum(gsum_col[:], gsum_part[:], axis=AX.X)
            sq_ps = psA.tile([C, 1], F32, tag="pa")
            nc.tensor.matmul(sq_ps[:], convwT[:], gsum_col[:], start=True,
                             stop=True)
            sq = singles.tile([C, 1], F32)
            nc.vector.tensor_scalar(sq[:], sq_ps[:], 1.0 / N, convb_c,
                                    ALU.mult, ALU.add)
            f1_ps = psA.tile([H, 1], F32, tag="pa")
            nc.tensor.matmul(f1_ps[:], fc1wT[:], sq[:], start=True, stop=True)
            f1 = singles.tile([H, 1], F32)
            nc.scalar.activation(f1[:], f1_ps[:], AF.Gelu, bias=fc1b_c)
            f2_ps = psA.tile([C, 1], F32, tag="pa")
            nc.tensor.matmul(f2_ps[:], fc2wT[:], f1[:], start=True, stop=True)
            # sigmoid(z) = 0.5*tanh(z/2) + 0.5 (tanh is in the gelu table)
            f2 = singles.tile([C, 1], F32)
            nc.scalar.activation(f2[:], f2_ps[:], AF.Tanh, scale=0.5,
                                 bias=halffc2b_c)
            nc.vector.tensor_scalar(f2[:], f2[:], 0.5, 0.5, ALU.mult, ALU.add)
            fb = singles.tile([C, 1], F32)     # f2 * conv0_b
            nc.vector.tensor_tensor(fb[:], f2[:], convb_c, ALU.mult)
            # W''[c, o] = convwT[c, o] * f2[o]  via  conv0_w @ diag(f2)
            diag = singles.tile([C, C], F32)
            nc.vector.tensor_scalar(diag[:], ident[:], f2[:], None, ALU.mult)
            wpp_ps = psA.tile([C, C], F32, tag="pa")
            nc.tensor.matmul(wpp_ps[:], wsb["conv0_w"][:], diag[:], start=True,
                             stop=True)
            wpp = singles.tile([C, C], BF16)
            nc.vector.tensor_copy(wpp[:], wpp_ps[:])

            # pass F: conv + bias-row matmul -> copy -> store
            for r in range(N // CH):
                off = r * CH
                cv_ps = psC.tile([C, CH], F32, tag="pc")
                for h in range(CH // 512):
                    o2 = off + h * 512
                    nc.tensor.matmul(cv_ps[:, h * 512:(h + 1) * 512], wpp[:],
                                     g[:, o2:o2 + 512], start=True, stop=True)
                ot = och.tile([C, CH], BF16, tag="ot")
                if r % 2 == 0:
                    nc.vector.tensor_scalar(ot[:], cv_ps[:], fb[:], None,
                                            ALU.add)
                else:
                    nc.scalar.activation(ot[:], cv_ps[:], AF.Identity,
                                         bias=fb[:])
                nc.sync.dma_start(out_d.ap()[:, off:off + CH], ot[:])


_NC_CACHE = {}


def _get_nc():
    if "nc" not in _NC_CACHE:
        _NC_CACHE["nc"] = build_nc()
    return _NC_CACHE["nc"]


def kernel(**inputs):
    x = np.ascontiguousarray(np.asarray(inputs["x"], dtype=np.float32))
    logits = np.ascontiguousarray(np.asarray(inputs["logits"],
                                             dtype=np.float32))
    assert x.shape == (B, C, N, 1) and logits.shape == (B, N)
    ident = np.eye(C, dtype=np.float32)
    shared = {"ident": ident}
    for nm in ("Wq1", "Wk1", "Wv1", "Wq2", "Wk2", "Wv2", "Wq3", "Wk3", "Wv3",
               "conv0_w", "fc1_w", "fc2_w", "ln_w", "ln_b", "bn_w", "bn_b",
               "conv0_b", "fc1_b", "fc2_b"):
        shared[nm] = np.ascontiguousarray(np.asarray(inputs[nm],
                                                     dtype=np.float32))
    in_maps = []
    for i in range(NCORES):
        m = dict(shared)
        m["x"] = np.ascontiguousarray(x[i, :, :, 0])
        m["logits"] = np.ascontiguousarray(logits[i])
        in_maps.append(m)

    nc = _get_nc()
    res = run_bass_kernel_spmd(nc, in_maps, list(range(NCORES))).results
    out = np.stack([res[i]["out"] for i in range(NCORES)], axis=0)
    return out[..., None].astype(np.float32)


# revision 5
# speedup vs baseline: 1.1315x; 1.1224x over previous
"""Trainium2 Bass kernel for nn_GSA_74045236183284 (histogram_binning), v2.

Sharding: data-parallel over batch B=8 across 8 NeuronCores (1 sample/core),
all params replicated, zero collectives (BatchNorm batch-stats coupling
approximated per-sample; validated far below tolerance).

Structure (per core):
  load x chunks (sync queue, head of line) -> cast to bf16 (vector/gpsimd)
  -> PE transposes (bf16) -> pooled bin sums via accumulating matmuls with
  strided bf16 mask weights; x^2 sums on scalar (Square+accum).
  Attention x3 with transposed-attn trick (softmax denominator folded in as
  a 9th V column), LN rsqrt via Newton iteration on the vector engine (no
  act-table thrash: only exp_and_others + gelu_and_others ever load).
  Closed-form instance/batch-norm stats in column orientation.
  Pass R: scatter matmul + identity-matmul x-add (PSUM accumulate) -> gelu
  straight from PSUM -> g (bf16) + gsum.  SE gates; f2 folded into conv
  weights via diag(f2) matmul; sigmoid via tanh identity.
  Pass F: conv matmul + bias-row matmul -> copy -> store.
"""

import sys

for _p in ("/opt/trn_rl_repo",):
    if _p not in sys.path:
        sys.path.insert(0, _p)

import numpy as np

import concourse.bass as bass
import concourse.bacc as bacc
import concourse.mybir as mybir
import concourse.tile as tile
from concourse.bass_utils import run_bass_kernel_spmd

F32 = mybir.dt.float32
BF16 = mybir.dt.bfloat16
U32 = mybir.dt.uint32
AF = mybir.ActivationFunctionType
ALU = mybir.AluOpType
AX = mybir.AxisListType

B, C, N, K = 8, 128, 16384, 8
NCORES = 8
LOADCH = 1024   # x load chunk (16 chunks)
CH = 1024       # pass R / pass F chunk (16 chunks)
H = C // 2


def build_nc():
    nc = bacc.Bacc("TRN2", target_bir_lowering=False, debug=False,
                   num_devices=NCORES)

    x_d = nc.dram_tensor("x", [C, N], F32, kind="ExternalInput")
    logits_d = nc.dram_tensor("logits", [N], F32, kind="ExternalInput")
    ident_d = nc.dram_tensor("ident", [C, C], F32, kind="ExternalInput")
    w_d = {}
    for nm in ("Wq1", "Wk1", "Wv1", "Wq2", "Wk2", "Wv2", "Wq3", "Wk3", "Wv3",
               "conv0_w"):
        w_d[nm] = nc.dram_tensor(nm, [C, C], F32, kind="ExternalInput")
    fc1w_d = nc.dram_tensor("fc1_w", [H, C], F32, kind="ExternalInput")
    fc2w_d = nc.dram_tensor("fc2_w", [C, H], F32, kind="ExternalInput")
    vecs = {}
    for nm in ("ln_w", "ln_b", "bn_w", "bn_b", "conv0_b", "fc2_b"):
        vecs[nm] = nc.dram_tensor(nm, [C], F32, kind="ExternalInput")
    vecs["fc1_b"] = nc.dram_tensor("fc1_b", [H], F32, kind="ExternalInput")
    out_d = nc.dram_tensor("out", [C, N], BF16, kind="ExternalOutput")

    with tile.TileContext(nc) as tc:
        _body(tc, nc, x_d, logits_d, ident_d, w_d, fc1w_d, fc2w_d, vecs, out_d)

    nc.compile()
    return nc


def _body(tc, nc, x_d, logits_d, ident_d, w_d, fc1w_d, fc2w_d, vecs, out_d):
    from contextlib import ExitStack
    ctx = ExitStack()
    with ctx:
        singles = ctx.enter_context(tc.tile_pool(name="singles", bufs=1))
        xload = ctx.enter_context(tc.tile_pool(name="xload", bufs=6))
        och = ctx.enter_context(tc.tile_pool(name="och", bufs=3))
        scr2 = ctx.enter_context(tc.tile_pool(name="scr2", bufs=2))
        dramp = ctx.enter_context(tc.tile_pool(name="dramp", bufs=1,
                                               space="DRAM"))
        psA = ctx.enter_context(tc.tile_pool(name="psA", bufs=1, space="PSUM"))
        psD_cm = tc.tile_pool(name="psD", bufs=2, space="PSUM")
        psD = psD_cm.__enter__()

        # ---------------- sync queue head: ident + all x chunk loads -------
        ident = singles.tile([C, C], F32)
        nc.sync.dma_start(ident[:], ident_d.ap())
        xts = []
        for ci in range(N // LOADCH):
            xt = xload.tile([C, LOADCH], F32, tag="xt")
            nc.sync.dma_start(xt[:], x_d.ap()[:, ci * LOADCH:(ci + 1) * LOADCH])
            xts.append(xt)

        # ---------------- small loads on gpsimd queue ----------------
        lg = singles.tile([C, C], F32)   # logits as [p, f], n = p*128+f
        nc.gpsimd.dma_start(lg[:], logits_d.ap().rearrange("(p f) -> p f", f=C))
        wsb = {}
        for nm in w_d:
            t = singles.tile([C, C], F32, tag=f"wl_{nm}")
            nc.gpsimd.dma_start(t[:], w_d[nm].ap())
            wsb[nm] = t
        fc1w = singles.tile([H, C], F32)
        nc.gpsimd.dma_start(fc1w[:], fc1w_d.ap())
        fc2w = singles.tile([C, H], F32)
        nc.gpsimd.dma_start(fc2w[:], fc2w_d.ap())
        vrows = singles.tile([7, C], F32)
        nc.vector.memset(vrows[:], 0.0)
        for r, nm in enumerate(("ln_w", "ln_b", "conv0_b", "bn_w", "bn_b")):
            nc.gpsimd.dma_start(vrows[r:r + 1, :], vecs[nm].ap()[None, :])
        nc.gpsimd.dma_start(vrows[5:6, :H], vecs["fc1_b"].ap()[None, :])
        nc.gpsimd.dma_start(vrows[6:7, :], vecs["fc2_b"].ap()[None, :])

        # ---------------- constants ----------------
        identb = singles.tile([C, C], BF16)
        nc.vector.tensor_copy(identb[:], ident[:])
        ones_col = singles.tile([C, 1], F32)
        nc.vector.memset(ones_col[:], 1.0)
        ones_row = singles.tile([1, C], F32)
        nc.vector.memset(ones_row[:], 1.0)
        onesCH_row = singles.tile([1, CH], BF16)
        nc.vector.memset(onesCH_row[:], 1.0)
        ones8 = singles.tile([K, 1], F32)
        nc.vector.memset(ones8[:], 1.0)
        magic = singles.tile([C, K], U32)
        nc.vector.memset(magic[:], 0x5f3759df)
        dummy = singles.tile([1, 1], F32)
        nc.vector.memset(dummy[:], 1.0)

        def rsqrt_newton(v_ap, out_ap, scr_ap, p, w, iters=2):
            # out = 1/sqrt(v) via bit-trick + Newton steps (vector only)
            ou = out_ap.bitcast(U32)
            nc.vector.tensor_scalar(ou, v_ap.bitcast(U32), 1, None,
                                    ALU.logical_shift_right)
            nc.vector.tensor_tensor(ou, magic[:p, :w], ou, ALU.subtract)
            for _ in range(iters):
                nc.vector.tensor_tensor(scr_ap, out_ap, out_ap, ALU.mult)
                nc.vector.tensor_tensor(scr_ap, scr_ap, v_ap, ALU.mult)
                nc.vector.tensor_scalar(scr_ap, scr_ap, -0.5, 1.5,
                                        ALU.mult, ALU.add)
                nc.vector.tensor_tensor(out_ap, out_ap, scr_ap, ALU.mult)

        # ---------------- masks from tanh(logits) (fp32 compares) ---------
        wA = singles.tile([C, C], F32)
        nc.scalar.activation(wA[:], lg[:], AF.Tanh)

        def build_masks(dst, src, nbins, eng):
            for j in range(8):
                lo = -1.0 + 0.25 * j
                eng.tensor_scalar(dst[:, j * C:(j + 1) * C], src[:],
                                  float(lo), None, ALU.is_gt)
            for j in range(7):
                eng.tensor_tensor(dst[:, j * C:(j + 1) * C],
                                  dst[:, j * C:(j + 1) * C],
                                  dst[:, (j + 1) * C:(j + 2) * C],
                                  ALU.subtract)
            neq = scr2.tile([C, C], dst.dtype, tag="neq" + eng.__class__.__name__)
            eng.tensor_scalar(neq[:], src[:], 0.0, None, ALU.not_equal)
            eng.tensor_tensor(dst[:, 3 * C:4 * C], dst[:, 3 * C:4 * C],
                              neq[:], ALU.mult)
            if nbins > 8:
                eng.memset(dst[:, 8 * C:9 * C], 1.0)

        # A-layout masks (bf16): [p, j*128+f], n = p*128+f
        mA = singles.tile([C, 8 * C], BF16)
        build_masks(mA, wA, 8, nc.vector)

        # Mrow [j, n] bf16 via DRAM roundtrip on the scalar HWDGE queue
        mrow_dram = dramp.tile([K, N], BF16)
        for j in range(K):
            nc.scalar.dma_start(mrow_dram[j:j + 1, :].rearrange("o n -> (o n)"),
                                mA[:, j * C:(j + 1) * C])
        Mrow = singles.tile([K, N], BF16)
        nc.scalar.dma_start(Mrow[:], mrow_dram[:])

        # nums: per-bin counts (exact: sums of 0/1)
        numsA = singles.tile([C, K], F32)
        for j in range(K):
            nc.vector.reduce_sum(numsA[:, j:j + 1], mA[:, j * C:(j + 1) * C],
                                 axis=AX.X)
        nums_ps = psD.tile([K, 1], F32, tag="pd")
        nc.tensor.matmul(nums_ps[:], numsA[:], ones_col[:], start=True,
                         stop=True)
        nums_c = singles.tile([K, 1], F32)
        nc.vector.tensor_copy(nums_c[:], nums_ps[:])
        rnums9_c = singles.tile([K + 1, 1], F32)
        nc.vector.memset(rnums9_c[:], 1.0)
        nc.vector.tensor_scalar(rnums9_c[:K], nums_c[:], 1.0, None, ALU.max)
        nc.vector.reciprocal(rnums9_c[:K], rnums9_c[:K])

        # T-layout masks (bf16) from wT: [i, j*128+q] = mask_j(n=q*128+i);
        # block j=8 is ones (sum-of-x row)
        wT_ps = psA.tile([C, C], F32, tag="pa")
        nc.tensor.transpose(wT_ps[:], wA[:], ident[:])
        wT = singles.tile([C, C], F32)
        nc.scalar.copy(wT[:], wT_ps[:])
        mT = singles.tile([C, 9 * C], BF16)
        build_masks(mT, wT, 9, nc.vector)

        # weight transposes (fold 1/temp into WqT)
        temp = float(np.sqrt(np.float32(C)))
        wqkT = []
        wvT = []
        for l in range(3):
            qk = singles.tile([C, 2 * C], F32, tag=f"wqkT{l}")
            ps = psA.tile([C, C], F32, tag="pa")
            nc.tensor.transpose(ps[:], wsb[f"Wq{l+1}"][:], ident[:])
            nc.scalar.activation(qk[:, :C], ps[:], AF.Copy, scale=1.0 / temp)
            ps = psA.tile([C, C], F32, tag="pa")
            nc.tensor.transpose(ps[:], wsb[f"Wk{l+1}"][:], ident[:])
            nc.scalar.copy(qk[:, C:], ps[:])
            wqkT.append(qk)
            vt = singles.tile([C, C], F32, tag=f"wvT{l}")
            ps = psA.tile([C, C], F32, tag="pa")
            nc.tensor.transpose(ps[:], wsb[f"Wv{l+1}"][:], ident[:])
            nc.scalar.copy(vt[:], ps[:])
            wvT.append(vt)
        convwT = singles.tile([C, C], F32)
        ps = psA.tile([C, C], F32, tag="pa")
        nc.tensor.transpose(ps[:], wsb["conv0_w"][:], ident[:])
        nc.scalar.copy(convwT[:], ps[:])
        fc1wT = singles.tile([C, H], F32)
        ps = psA.tile([C, C], F32, tag="pa")
        nc.tensor.transpose(ps[:, :H], fc1w[:], ident[:H, :H])
        nc.scalar.copy(fc1wT[:], ps[:, :H])
        fc2wT = singles.tile([H, C], F32)
        ps = psA.tile([C, C], F32, tag="pa")
        nc.tensor.transpose(ps[:H, :], fc2w[:], ident[:])
        nc.scalar.copy(fc2wT[:], ps[:H, :])
        ps = psA.tile([C, C], F32, tag="pa")
        nc.tensor.transpose(ps[:, :7], vrows[:], ident[:7, :7])
        vcols = singles.tile([C, 7], F32)
        nc.scalar.copy(vcols[:], ps[:, :7])
        lnw_c, lnb_c = vcols[:, 0:1], vcols[:, 1:2]
        convb_c = vcols[:, 2:3]
        bnw_c, bnb_c = vcols[:, 3:4], vcols[:, 4:5]
        fc1b_c = vcols[:H, 5:6]
        halffc2b_c = singles.tile([C, 1], F32)
        nc.vector.tensor_scalar(halffc2b_c[:], vcols[:, 6:7], 0.5, None,
                                ALU.mult)

        # ---------------- x: cast -> PE transpose -> pooled ----------------
        xb = singles.tile([C, N], BF16)      # x in bf16, [c, n]
        xT = singles.tile([C, N], BF16)      # x^T tiles: [i, q*128+c]
        xsq_part = singles.tile([C, N // LOADCH], F32)
        psB_cm = tc.tile_pool(name="psB", bufs=1, space="PSUM")
        psB = psB_cm.__enter__()
        pooled_ps = psB.tile([K + 1, C], F32)
        with tc.tile_pool(name="psT", bufs=4, space="PSUM") as psT:
            for ci in range(N // LOADCH):
                xt = xts[ci]
                xbs = xb[:, ci * LOADCH:(ci + 1) * LOADCH]
                nc.vector.tensor_copy(xbs, xt[:])

                for grp in range(2):
                    q0 = ci * (LOADCH // C) + grp * 4
                    pt = psT.tile([C, 4 * C], BF16, tag="pt")
                    for g_ in range(4):
                        q = q0 + g_
                        nc.tensor.transpose(pt[:, g_ * C:(g_ + 1) * C],
                                            xb[:, q * C:(q + 1) * C],
                                            identb[:])
                    dst = xT[:, q0 * C:(q0 + 4) * C]
                    nc.scalar.copy(dst, pt[:])
                    for g_ in range(4):
                        q = q0 + g_
                        nc.tensor.matmul(pooled_ps[:], mT[:, q::C],
                                         xT[:, q * C:(q + 1) * C],
                                         start=(q == 0),
                                         stop=(q == N // C - 1))

        # pooledT9: rows 0-7 per-bin sums, row 8 = sum_n x
        pooledT9 = singles.tile([K + 1, C], F32)
        nc.vector.tensor_copy(pooledT9[:], pooled_ps[:])
        pooledT = pooledT9[:K, :]
        psB_cm.__exit__(None, None, None)

        # combo = [featT rows; sumx row] in one op (rnums9 row 8 = 1.0)
        combo = singles.tile([K + 1, C], F32)
        nc.vector.tensor_scalar(combo[:], pooledT9[:], rnums9_c[:], None,
                                ALU.mult)
        cps = psA.tile([C, C], F32, tag="pa")
        nc.tensor.transpose(cps[:, :K + 1], combo[:], ident[:K + 1, :K + 1])
        fea9 = singles.tile([C, K + 1], F32)
        nc.vector.tensor_copy(fea9[:], cps[:, :K + 1])
        fea = fea9[:, :K]
        sumx_col = fea9[:, K:K + 1]

        # x^2 channel sums on scalar (scheduler overlaps these with the
        # pooled tail / attention window)
        for ci in range(N // LOADCH):
            sqscr = scr2.tile([C, LOADCH], BF16, tag="sqscr")
            nc.scalar.activation(sqscr[:], xb[:, ci * LOADCH:(ci + 1) * LOADCH],
                                 AF.Square, accum_out=xsq_part[:, ci:ci + 1])

        # ---------------- attention x3 ----------------
        for l in range(3):
            qk_ps = psD.tile([K, 2 * C], F32, tag="pd")
            nc.tensor.matmul(qk_ps[:], fea, wqkT[l][:], start=True, stop=True)
            qkT = singles.tile([K, 2 * C], F32, tag=f"qkT{l}")
            nc.vector.tensor_copy(qkT[:], qk_ps[:])
            v_ps = psD.tile([C, K], F32, tag="pd")
            nc.tensor.matmul(v_ps[:], wvT[l][:], fea, start=True, stop=True)
            v9 = singles.tile([C, K + 1], F32, tag=f"v9{l}")
            nc.vector.tensor_copy(v9[:, :K], v_ps[:])
            nc.vector.memset(v9[:, K:K + 1], 1.0)
            # attn^T directly: atT[d, c] = sum_j k[d,j] q[c,j]
            atT_ps = psA.tile([C, C], F32, tag="pa")
            nc.tensor.matmul(atT_ps[:], qkT[:, C:], qkT[:, :C], start=True,
                             stop=True)
            expT = singles.tile([C, C], F32, tag=f"eT{l}")
            nc.scalar.activation(expT[:], atT_ps[:], AF.Exp)
            # ao9 = expT^T @ v9: cols 0-7 unnormalized out, col 8 = denom
            ao9_ps = psD.tile([C, K + 1], F32, tag="pd")
            nc.tensor.matmul(ao9_ps[:], expT[:], v9[:], start=True, stop=True)
            rse = singles.tile([C, 1], F32, tag=f"rse{l}")
            nc.vector.reciprocal(rse[:], ao9_ps[:, K:K + 1])
            # osb = ao9*rse + fea (residual); stin = [osb | osb^2]
            stin = singles.tile([C, 2 * K], F32, tag=f"stin{l}")
            nc.vector.tensor_scalar(stin[:, :K], ao9_ps[:, :K], rse[:], None,
                                    ALU.mult)
            nc.vector.tensor_tensor(stin[:, :K], stin[:, :K], fea, ALU.add)
            nc.vector.tensor_tensor(stin[:, K:], stin[:, :K], stin[:, :K],
                                    ALU.mult)
            # LN over c (partition dim): st = ones^T @ stin -> [1, 16]
            st_ps = psD.tile([1, 2 * K], F32, tag="pd")
            nc.tensor.matmul(st_ps[:], ones_col[:], stin[:], start=True,
                             stop=True)
            mr = singles.tile([1, 2 * K], F32, tag=f"mr{l}")
            nc.vector.tensor_scalar(mr[:], st_ps[:], 1.0 / C, None, ALU.mult)
            # var = E[x^2] - mu^2 ; rs = 1/sqrt(var+eps) via newton (vector)
            vs8 = singles.tile([1, 2 * K], F32, tag=f"vs8{l}")
            nc.vector.tensor_tensor(vs8[:, :K], mr[:, :K], mr[:, :K], ALU.mult)
            nc.vector.tensor_tensor(vs8[:, :K], mr[:, K:], vs8[:, :K],
                                    ALU.subtract)
            nc.vector.tensor_scalar(vs8[:, :K], vs8[:, :K], 1e-6, None,
                                    ALU.add)
            rsqrt_newton(vs8[:, :K], mr[:, K:], vs8[:, K:], 1, K, iters=1)
            # broadcast [mu | rs] down partitions
            bc_ps = psD.tile([C, 2 * K], F32, tag="pd")
            nc.tensor.matmul(bc_ps[:], ones_row[:], mr[:], start=True,
                             stop=True)
            fea2 = singles.tile([C, K], F32, tag=f"fea{l+1}")
            nc.vector.tensor_tensor(fea2[:], stin[:, :K], bc_ps[:, :K],
                                    ALU.subtract)
            nc.vector.tensor_tensor(fea2[:], fea2[:], bc_ps[:, K:], ALU.mult)
            nc.vector.tensor_scalar(fea2[:], fea2[:], lnw_c, lnb_c,
                                    ALU.mult, ALU.add)
            fea = fea2[:]

        # ---------------- closed-form instance/batch-norm stats -----------
        ftp = psA.tile([C, C], F32, tag="pa")
        nc.tensor.transpose(ftp[:K, :], fea, ident[:])
        featb = singles.tile([K, C], BF16)
        nc.vector.tensor_copy(featb[:], ftp[:K, :])
        featr = singles.tile([K, C], F32)
        nc.vector.tensor_copy(featr[:], featb[:])
        fsq = singles.tile([K, C], F32)
        nc.vector.tensor_tensor(fsq[:], featr[:], featr[:], ALU.mult)
        prod = singles.tile([K, C], F32)
        nc.vector.tensor_tensor(prod[:], featr[:], pooledT, ALU.mult)
        r1_ps = psD.tile([C, 1], F32, tag="pd")
        nc.tensor.matmul(r1_ps[:], featr[:], nums_c[:], start=True, stop=True)
        r2_ps = psD.tile([C, 1], F32, tag="pd")
        nc.tensor.matmul(r2_ps[:], fsq[:], nums_c[:], start=True, stop=True)
        r3_ps = psA.tile([C, 1], F32, tag="pa")
        nc.tensor.matmul(r3_ps[:], prod[:], ones8[:], start=True, stop=True)

        xsq_col = singles.tile([C, 1], F32)
        nc.vector.reduce_sum(xsq_col[:], xsq_part[:], axis=AX.X)
        stats = singles.tile([C, 8], F32)
        mu = stats[:, 0:1]
        var = stats[:, 1:2]
        rsb = stats[:, 2:3]
        s_col = stats[:, 3:4]
        b_col = stats[:, 4:5]
        tmp = stats[:, 5:6]
        nc.vector.tensor_tensor(tmp[:], sumx_col, r1_ps[:], ALU.add)
        nc.vector.tensor_scalar(mu[:], tmp[:], 1.0 / N, None, ALU.mult)
        nc.vector.tensor_scalar(tmp[:], r3_ps[:], 2.0, None, ALU.mult)
        nc.vector.tensor_tensor(tmp[:], tmp[:], r2_ps[:], ALU.add)
        nc.vector.tensor_tensor(tmp[:], tmp[:], xsq_col[:], ALU.add)
        nc.vector.tensor_scalar(tmp[:], tmp[:], 1.0 / N, None, ALU.mult)
        nc.vector.tensor_tensor(var[:], mu[:], mu[:], ALU.mult)
        nc.vector.tensor_tensor(var[:], tmp[:], var[:], ALU.subtract)
        # rs_i = 1/sqrt(var+1e-5); vb = var/(var+1e-5); rs_b = 1/sqrt(vb+1e-5)
        rsi = stats[:, 6:7]
        inv = stats[:, 7:8]
        scr2c = singles.tile([C, 2], F32, tag="nsc")
        nc.vector.tensor_scalar(inv[:], var[:], 1e-5, None, ALU.add)
        rsqrt_newton(inv[:], rsi[:], scr2c[:, 0:1], C, 1)
        nc.vector.reciprocal(inv[:], inv[:])
        nc.vector.tensor_tensor(rsb[:], var[:], inv[:], ALU.mult)
        nc.vector.tensor_scalar(rsb[:], rsb[:], 1e-5, None, ALU.add)
        rsqrt_newton(rsb[:], scr2c[:, 1:2], scr2c[:, 0:1], C, 1)
        nc.vector.tensor_copy(rsb[:], scr2c[:, 1:2])
        # dummy gelu: pull the gelu-table load off the critical path
        nc.scalar.activation(dummy[:], dummy[:], AF.Gelu)
        nc.vector.tensor_tensor(s_col[:], rsi[:], rsb[:], ALU.mult)
        nc.vector.tensor_tensor(s_col[:], s_col[:], bnw_c, ALU.mult)
        nc.vector.tensor_tensor(b_col[:], mu[:], s_col[:], ALU.mult)
        nc.vector.tensor_tensor(b_col[:], bnb_c, b_col[:], ALU.subtract)

        psD_cm.__exit__(None, None, None)

        # ---------------- pass R + gates + pass F ----------------
        g = singles.tile([C, N], BF16)
        gsum_part = singles.tile([C, N // CH], F32)
        with tc.tile_pool(name="psC", bufs=3, space="PSUM") as psC:
            for r in range(N // CH):
                off = r * CH
                sc_ps = psC.tile([C, CH], F32, tag="pc")
                for h in range(CH // 512):
                    o2 = off + h * 512
                    sl = sc_ps[:, h * 512:(h + 1) * 512]
                    nc.tensor.matmul(sl, identb[:], xb[:, o2:o2 + 512],
                                     start=True, stop=False)
                    nc.tensor.matmul(sl, featb[:], Mrow[:, o2:o2 + 512],
                                     start=False, stop=True)
                nc.scalar.activation(g[:, off:off + CH], sc_ps[:], AF.Gelu,
                                     bias=b_col, scale=s_col,
                                     accum_out=gsum_part[:, r:r + 1])

            # SE gates
            gsum_col = singles.tile([C, 1], F32)
            nc.vector.reduce_sum(gsum_col[:], gsum_part[:], axis=AX.X)
            sq_ps = psA.tile([C, 1], F32, tag="pa")
            nc.tensor.matmul(sq_ps[:], convwT[:], gsum_col[:], start=True,
                             stop=True)
            sq = singles.tile([C, 1], F32)
            nc.vector.tensor_scalar(sq[:], sq_ps[:], 1.0 / N, convb_c,
                                    ALU.mult, ALU.add)
            f1_ps = psA.tile([H, 1], F32, tag="pa")
            nc.tensor.matmul(f1_ps[:], fc1wT[:], sq[:], start=True, stop=True)
            f1 = singles.tile([H, 1], F32)
            nc.scalar.activation(f1[:], f1_ps[:], AF.Gelu, bias=fc1b_c)
            f2_ps = psA.tile([C, 1], F32, tag="pa")
            nc.tensor.matmul(f2_ps[:], fc2wT[:], f1[:], start=True, stop=True)
            # sigmoid(z) = 0.5*tanh(z/2) + 0.5 (tanh is in the gelu table)
            f2 = singles.tile([C, 1], F32)
            nc.scalar.activation(f2[:], f2_ps[:], AF.Tanh, scale=0.5,
                                 bias=halffc2b_c)
            nc.vector.tensor_scalar(f2[:], f2[:], 0.5, 0.5, ALU.mult, ALU.add)
            fb = singles.tile([C, 1], F32)     # f2 * conv0_b
            nc.vector.tensor_tensor(fb[:], f2[:], convb_c, ALU.mult)
            # W''[c, o] = convwT[c, o] * f2[o]  via  conv0_w @ diag(f2)
            diag = singles.tile([C, C], F32)
            nc.vector.tensor_scalar(diag[:], ident[:], f2[:], None, ALU.mult)
            wpp_ps = psA.tile([C, C], F32, tag="pa")
            nc.tensor.matmul(wpp_ps[:], wsb["conv0_w"][:], diag[:], start=True,
                             stop=True)
            wpp = singles.tile([C, C], BF16)
            nc.vector.tensor_copy(wpp[:], wpp_ps[:])

            # pass F: conv + bias-row matmul -> copy -> store
            for r in range(N // CH):
                off = r * CH
                cv_ps = psC.tile([C, CH], F32, tag="pc")
                for h in range(CH // 512):
                    o2 = off + h * 512
                    nc.tensor.matmul(cv_ps[:, h * 512:(h + 1) * 512], wpp[:],
                                     g[:, o2:o2 + 512], start=True, stop=True)
                ot = och.tile([C, CH], BF16, tag="ot")
                if r % 2 == 0:
                    nc.vector.tensor_scalar(ot[:], cv_ps[:], fb[:], None,
                                            ALU.add)
                else:
                    nc.scalar.activation(ot[:], cv_ps[:], AF.Identity,
                                         bias=fb[:])
                nc.sync.dma_start(out_d.ap()[:, off:off + CH], ot[:])


_NC_CACHE = {}


def _get_nc():
    if "nc" not in _NC_CACHE:
        _NC_CACHE["nc"] = build_nc()
    return _NC_CACHE["nc"]


def kernel(**inputs):
    x = np.ascontiguousarray(np.asarray(inputs["x"], dtype=np.float32))
    logits = np.ascontiguousarray(np.asarray(inputs["logits"],
                                             dtype=np.float32))
    assert x.shape == (B, C, N, 1) and logits.shape == (B, N)
    ident = np.eye(C, dtype=np.float32)
    shared = {"ident": ident}
    for nm in ("Wq1", "Wk1", "Wv1", "Wq2", "Wk2", "Wv2", "Wq3", "Wk3", "Wv3",
               "conv0_w", "fc1_w", "fc2_w", "ln_w", "ln_b", "bn_w", "bn_b",
               "conv0_b", "fc1_b", "fc2_b"):
        shared[nm] = np.ascontiguousarray(np.asarray(inputs[nm],
                                                     dtype=np.float32))
    in_maps = []
    for i in range(NCORES):
        m = dict(shared)
        m["x"] = np.ascontiguousarray(x[i, :, :, 0])
        m["logits"] = np.ascontiguousarray(logits[i])
        in_maps.append(m)

    nc = _get_nc()
    res = run_bass_kernel_spmd(nc, in_maps, list(range(NCORES))).results
    out = np.stack([res[i]["out"] for i in range(NCORES)], axis=0)
    return out[..., None].astype(np.float32)


# revision 6
# speedup vs baseline: 1.2003x; 1.0607x over previous
"""Trainium2 Bass kernel for nn_GSA_74045236183284 (histogram_binning), v2.

Sharding: data-parallel over batch B=8 across 8 NeuronCores (1 sample/core),
all params replicated, zero collectives (BatchNorm batch-stats coupling
approximated per-sample; validated far below tolerance).

Structure (per core):
  load x chunks (sync queue, head of line) -> cast to bf16 (vector/gpsimd)
  -> PE transposes (bf16) -> pooled bin sums via accumulating matmuls with
  strided bf16 mask weights; x^2 sums on scalar (Square+accum).
  Attention x3 with transposed-attn trick (softmax denominator folded in as
  a 9th V column), LN rsqrt via Newton iteration on the vector engine (no
  act-table thrash: only exp_and_others + gelu_and_others ever load).
  Closed-form instance/batch-norm stats in column orientation.
  Pass R: scatter matmul + identity-matmul x-add (PSUM accumulate) -> gelu
  straight from PSUM -> g (bf16) + gsum.  SE gates; f2 folded into conv
  weights via diag(f2) matmul; sigmoid via tanh identity.
  Pass F: conv matmul + bias-row matmul -> copy -> store.
"""

import sys

for _p in ("/opt/trn_rl_repo",):
    if _p not in sys.path:
        sys.path.insert(0, _p)

import numpy as np

import concourse.bass as bass
import concourse.bacc as bacc
import concourse.mybir as mybir
import concourse.tile as tile
from concourse.bass_utils import run_bass_kernel_spmd

F32 = mybir.dt.float32
BF16 = mybir.dt.bfloat16
U32 = mybir.dt.uint32
AF = mybir.ActivationFunctionType
ALU = mybir.AluOpType
AX = mybir.AxisListType

B, C, N, K = 8, 128, 16384, 8
NCORES = 8
LOADCH = 1024   # x load chunk (16 chunks)
CH = 1024       # pass R / pass F chunk (16 chunks)
H = C // 2


def build_nc():
    nc = bacc.Bacc("TRN2", target_bir_lowering=False, debug=False,
                   num_devices=NCORES)

    x_d = nc.dram_tensor("x", [C, N], F32, kind="ExternalInput")
    logits_d = nc.dram_tensor("logits", [N], F32, kind="ExternalInput")
    ident_d = nc.dram_tensor("ident", [C, C], F32, kind="ExternalInput")
    w_d = {}
    for nm in ("Wq1", "Wk1", "Wv1", "Wq2", "Wk2", "Wv2", "Wq3", "Wk3", "Wv3",
               "conv0_w"):
        w_d[nm] = nc.dram_tensor(nm, [C, C], F32, kind="ExternalInput")
    fc1w_d = nc.dram_tensor("fc1_w", [H, C], F32, kind="ExternalInput")
    fc2w_d = nc.dram_tensor("fc2_w", [C, H], F32, kind="ExternalInput")
    vecs = {}
    for nm in ("ln_w", "ln_b", "bn_w", "bn_b", "conv0_b", "fc2_b"):
        vecs[nm] = nc.dram_tensor(nm, [C], F32, kind="ExternalInput")
    vecs["fc1_b"] = nc.dram_tensor("fc1_b", [H], F32, kind="ExternalInput")
    out_d = nc.dram_tensor("out", [C, N], BF16, kind="ExternalOutput")

    with tile.TileContext(nc) as tc:
        _body(tc, nc, x_d, logits_d, ident_d, w_d, fc1w_d, fc2w_d, vecs, out_d)

    nc.compile()
    return nc


def _body(tc, nc, x_d, logits_d, ident_d, w_d, fc1w_d, fc2w_d, vecs, out_d):
    from contextlib import ExitStack
    ctx = ExitStack()
    with ctx:
        singles = ctx.enter_context(tc.tile_pool(name="singles", bufs=1))
        xload = ctx.enter_context(tc.tile_pool(name="xload", bufs=6))
        och = ctx.enter_context(tc.tile_pool(name="och", bufs=3))
        scr2 = ctx.enter_context(tc.tile_pool(name="scr2", bufs=2))
        dramp = ctx.enter_context(tc.tile_pool(name="dramp", bufs=1,
                                               space="DRAM"))
        psA = ctx.enter_context(tc.tile_pool(name="psA", bufs=1, space="PSUM"))
        psD_cm = tc.tile_pool(name="psD", bufs=2, space="PSUM")
        psD = psD_cm.__enter__()

        # ---------------- sync queue head: ident + all x chunk loads -------
        ident = singles.tile([C, C], F32)
        nc.sync.dma_start(ident[:], ident_d.ap())
        xts = []
        for ci in range(N // LOADCH):
            xt = xload.tile([C, LOADCH], F32, tag="xt")
            nc.sync.dma_start(xt[:], x_d.ap()[:, ci * LOADCH:(ci + 1) * LOADCH])
            xts.append(xt)

        # ---------------- small loads on gpsimd queue ----------------
        lg = singles.tile([C, C], F32)   # logits as [p, f], n = p*128+f
        nc.gpsimd.dma_start(lg[:], logits_d.ap().rearrange("(p f) -> p f", f=C))
        wsb = {}
        for nm in w_d:
            t = singles.tile([C, C], F32, tag=f"wl_{nm}")
            nc.gpsimd.dma_start(t[:], w_d[nm].ap())
            wsb[nm] = t
        fc1w = singles.tile([H, C], F32)
        nc.gpsimd.dma_start(fc1w[:], fc1w_d.ap())
        fc2w = singles.tile([C, H], F32)
        nc.gpsimd.dma_start(fc2w[:], fc2w_d.ap())
        vrows = singles.tile([7, C], F32)
        nc.vector.memset(vrows[:], 0.0)
        for r, nm in enumerate(("ln_w", "ln_b", "conv0_b", "bn_w", "bn_b")):
            nc.gpsimd.dma_start(vrows[r:r + 1, :], vecs[nm].ap()[None, :])
        nc.gpsimd.dma_start(vrows[5:6, :H], vecs["fc1_b"].ap()[None, :])
        nc.gpsimd.dma_start(vrows[6:7, :], vecs["fc2_b"].ap()[None, :])

        # ---------------- constants ----------------
        identb = singles.tile([C, C], BF16)
        nc.vector.tensor_copy(identb[:], ident[:])
        ones_col = singles.tile([C, 1], F32)
        nc.vector.memset(ones_col[:], 1.0)
        ones_row = singles.tile([1, C], F32)
        nc.vector.memset(ones_row[:], 1.0)
        onesCH_row = singles.tile([1, CH], BF16)
        nc.vector.memset(onesCH_row[:], 1.0)
        ones8 = singles.tile([K, 1], F32)
        nc.vector.memset(ones8[:], 1.0)
        magic = singles.tile([C, K], U32)
        nc.vector.memset(magic[:], 0x5f3759df)
        dummy = singles.tile([1, 1], F32)
        nc.vector.memset(dummy[:], 1.0)

        def rsqrt_newton(v_ap, out_ap, scr_ap, p, w, iters=2):
            # out = 1/sqrt(v) via bit-trick + Newton steps (vector only)
            ou = out_ap.bitcast(U32)
            nc.vector.tensor_scalar(ou, v_ap.bitcast(U32), 1, None,
                                    ALU.logical_shift_right)
            nc.vector.tensor_tensor(ou, magic[:p, :w], ou, ALU.subtract)
            for _ in range(iters):
                nc.vector.tensor_tensor(scr_ap, out_ap, out_ap, ALU.mult)
                nc.vector.tensor_tensor(scr_ap, scr_ap, v_ap, ALU.mult)
                nc.vector.tensor_scalar(scr_ap, scr_ap, -0.5, 1.5,
                                        ALU.mult, ALU.add)
                nc.vector.tensor_tensor(out_ap, out_ap, scr_ap, ALU.mult)

        # ---------------- masks from tanh(logits) (fp32 compares) ---------
        wA = singles.tile([C, C], F32)
        nc.scalar.activation(wA[:], lg[:], AF.Tanh)

        def build_masks(dst, src, nbins, eng):
            for j in range(8):
                lo = -1.0 + 0.25 * j
                eng.tensor_scalar(dst[:, j * C:(j + 1) * C], src[:],
                                  float(lo), None, ALU.is_gt)
            for j in range(7):
                eng.tensor_tensor(dst[:, j * C:(j + 1) * C],
                                  dst[:, j * C:(j + 1) * C],
                                  dst[:, (j + 1) * C:(j + 2) * C],
                                  ALU.subtract)
            neq = scr2.tile([C, C], dst.dtype, tag="neq" + eng.__class__.__name__)
            eng.tensor_scalar(neq[:], src[:], 0.0, None, ALU.not_equal)
            eng.tensor_tensor(dst[:, 3 * C:4 * C], dst[:, 3 * C:4 * C],
                              neq[:], ALU.mult)
            if nbins > 8:
                eng.memset(dst[:, 8 * C:9 * C], 1.0)

        # A-layout masks (bf16): [p, j*128+f], n = p*128+f
        mA = singles.tile([C, 8 * C], BF16)
        build_masks(mA, wA, 8, nc.vector)

        # Mrow [j, n] bf16 via DRAM roundtrip on the scalar HWDGE queue
        mrow_dram = dramp.tile([K, N], BF16)
        for j in range(K):
            nc.scalar.dma_start(mrow_dram[j:j + 1, :].rearrange("o n -> (o n)"),
                                mA[:, j * C:(j + 1) * C])
        Mrow = singles.tile([K, N], BF16)
        nc.scalar.dma_start(Mrow[:], mrow_dram[:])

        # nums: per-bin counts (exact: sums of 0/1)
        numsA = singles.tile([C, K], F32)
        for j in range(K):
            nc.vector.reduce_sum(numsA[:, j:j + 1], mA[:, j * C:(j + 1) * C],
                                 axis=AX.X)
        nums_ps = psD.tile([K, 1], F32, tag="pd")
        nc.tensor.matmul(nums_ps[:], numsA[:], ones_col[:], start=True,
                         stop=True)
        nums_c = singles.tile([K, 1], F32)
        nc.vector.tensor_copy(nums_c[:], nums_ps[:])
        rnums9_c = singles.tile([K + 1, 1], F32)
        nc.vector.memset(rnums9_c[:], 1.0)
        nc.vector.tensor_scalar(rnums9_c[:K], nums_c[:], 1.0, None, ALU.max)
        nc.vector.reciprocal(rnums9_c[:K], rnums9_c[:K])

        # T-layout masks (bf16) from wT: [i, j*128+q] = mask_j(n=q*128+i);
        # block j=8 is ones (sum-of-x row)
        wT_ps = psA.tile([C, C], F32, tag="pa")
        nc.tensor.transpose(wT_ps[:], wA[:], ident[:])
        wT = singles.tile([C, C], F32)
        nc.scalar.copy(wT[:], wT_ps[:])
        mT = singles.tile([C, 9 * C], BF16)
        build_masks(mT, wT, 9, nc.vector)

        # weight transposes (fold 1/temp into WqT)
        temp = float(np.sqrt(np.float32(C)))
        wqkT = []
        wvT = []
        for l in range(3):
            qk = singles.tile([C, 2 * C], F32, tag=f"wqkT{l}")
            ps = psA.tile([C, C], F32, tag="pa")
            nc.tensor.transpose(ps[:], wsb[f"Wq{l+1}"][:], ident[:])
            nc.scalar.activation(qk[:, :C], ps[:], AF.Copy, scale=1.0 / temp)
            ps = psA.tile([C, C], F32, tag="pa")
            nc.tensor.transpose(ps[:], wsb[f"Wk{l+1}"][:], ident[:])
            nc.scalar.copy(qk[:, C:], ps[:])
            wqkT.append(qk)
            vt = singles.tile([C, C], F32, tag=f"wvT{l}")
            ps = psA.tile([C, C], F32, tag="pa")
            nc.tensor.transpose(ps[:], wsb[f"Wv{l+1}"][:], ident[:])
            nc.scalar.copy(vt[:], ps[:])
            wvT.append(vt)
        convwT = singles.tile([C, C], F32)
        ps = psA.tile([C, C], F32, tag="pa")
        nc.tensor.transpose(ps[:], wsb["conv0_w"][:], ident[:])
        nc.scalar.copy(convwT[:], ps[:])
        fc1wT = singles.tile([C, H], F32)
        ps = psA.tile([C, C], F32, tag="pa")
        nc.tensor.transpose(ps[:, :H], fc1w[:], ident[:H, :H])
        nc.scalar.copy(fc1wT[:], ps[:, :H])
        fc2wT = singles.tile([H, C], F32)
        ps = psA.tile([C, C], F32, tag="pa")
        nc.tensor.transpose(ps[:H, :], fc2w[:], ident[:])
        nc.scalar.copy(fc2wT[:], ps[:H, :])
        ps = psA.tile([C, C], F32, tag="pa")
        nc.tensor.transpose(ps[:, :7], vrows[:], ident[:7, :7])
        vcols = singles.tile([C, 7], F32)
        nc.scalar.copy(vcols[:], ps[:, :7])
        lnw_c, lnb_c = vcols[:, 0:1], vcols[:, 1:2]
        convb_c = vcols[:, 2:3]
        bnw_c, bnb_c = vcols[:, 3:4], vcols[:, 4:5]
        fc1b_c = vcols[:H, 5:6]
        halffc2b_c = singles.tile([C, 1], F32)
        nc.vector.tensor_scalar(halffc2b_c[:], vcols[:, 6:7], 0.5, None,
                                ALU.mult)

        # ---------------- x: cast -> PE transpose -> pooled ----------------
        xb = singles.tile([C, N], BF16)      # x in bf16, [c, n]
        xT = singles.tile([C, N], BF16)      # x^T tiles: [i, q*128+c]
        xsq_part = singles.tile([C, N // LOADCH], F32)
        psB_cm = tc.tile_pool(name="psB", bufs=1, space="PSUM")
        psB = psB_cm.__enter__()
        pooled_ps = psB.tile([K + 1, C], F32)
        with tc.tile_pool(name="psT", bufs=4, space="PSUM") as psT:
            for ci in range(N // LOADCH):
                xt = xts[ci]
                xbs = xb[:, ci * LOADCH:(ci + 1) * LOADCH]
                nc.vector.tensor_copy(xbs, xt[:])

                for grp in range(2):
                    q0 = ci * (LOADCH // C) + grp * 4
                    pt = psT.tile([C, 4 * C], BF16, tag="pt")
                    for g_ in range(4):
                        q = q0 + g_
                        nc.tensor.transpose(pt[:, g_ * C:(g_ + 1) * C],
                                            xb[:, q * C:(q + 1) * C],
                                            identb[:])
                    dst = xT[:, q0 * C:(q0 + 4) * C]
                    nc.scalar.copy(dst, pt[:])
                    for g_ in range(4):
                        q = q0 + g_
                        nc.tensor.matmul(pooled_ps[:], mT[:, q::C],
                                         xT[:, q * C:(q + 1) * C],
                                         start=(q == 0),
                                         stop=(q == N // C - 1))

        # pooledT9: rows 0-7 per-bin sums, row 8 = sum_n x
        pooledT9 = singles.tile([K + 1, C], F32)
        nc.vector.tensor_copy(pooledT9[:], pooled_ps[:])
        pooledT = pooledT9[:K, :]
        psB_cm.__exit__(None, None, None)

        # combo = [featT rows; sumx row] in one op (rnums9 row 8 = 1.0)
        combo = singles.tile([K + 1, C], F32)
        nc.vector.tensor_scalar(combo[:], pooledT9[:], rnums9_c[:], None,
                                ALU.mult)
        cps = psA.tile([C, C], F32, tag="pa")
        nc.tensor.transpose(cps[:, :K + 1], combo[:], ident[:K + 1, :K + 1])
        fea9 = singles.tile([C, K + 1], F32)
        nc.vector.tensor_copy(fea9[:], cps[:, :K + 1])
        fea = fea9[:, :K]
        sumx_col = fea9[:, K:K + 1]

        # ---------------- attention x3 ----------------
        for l in range(3):
            qk_ps = psD.tile([K, 2 * C], F32, tag="pd")
            nc.tensor.matmul(qk_ps[:], fea, wqkT[l][:], start=True, stop=True)
            qkT = singles.tile([K, 2 * C], F32, tag=f"qkT{l}")
            nc.vector.tensor_copy(qkT[:], qk_ps[:])
            v_ps = psD.tile([C, K], F32, tag="pd")
            nc.tensor.matmul(v_ps[:], wvT[l][:], fea, start=True, stop=True)
            v9 = singles.tile([C, K + 1], F32, tag=f"v9{l}")
            nc.vector.tensor_copy(v9[:, :K], v_ps[:])
            nc.vector.memset(v9[:, K:K + 1], 1.0)
            # attn^T directly: atT[d, c] = sum_j k[d,j] q[c,j]
            atT_ps = psA.tile([C, C], F32, tag="pa")
            nc.tensor.matmul(atT_ps[:], qkT[:, C:], qkT[:, :C], start=True,
                             stop=True)
            expT = singles.tile([C, C], F32, tag=f"eT{l}")
            nc.scalar.activation(expT[:], atT_ps[:], AF.Exp)
            # ao9 = expT^T @ v9: cols 0-7 unnormalized out, col 8 = denom
            ao9_ps = psD.tile([C, K + 1], F32, tag="pd")
            nc.tensor.matmul(ao9_ps[:], expT[:], v9[:], start=True, stop=True)
            rse = singles.tile([C, 1], F32, tag=f"rse{l}")
            nc.vector.reciprocal(rse[:], ao9_ps[:, K:K + 1])
            # osb = ao9*rse + fea (residual); stin = [osb | osb^2]
            stin = singles.tile([C, 2 * K], F32, tag=f"stin{l}")
            nc.vector.tensor_scalar(stin[:, :K], ao9_ps[:, :K], rse[:], None,
                                    ALU.mult)
            nc.vector.tensor_tensor(stin[:, :K], stin[:, :K], fea, ALU.add)
            nc.vector.tensor_tensor(stin[:, K:], stin[:, :K], stin[:, :K],
                                    ALU.mult)
            # LN over c (partition dim): st = ones^T @ stin -> [1, 16]
            st_ps = psD.tile([1, 2 * K], F32, tag="pd")
            nc.tensor.matmul(st_ps[:], ones_col[:], stin[:], start=True,
                             stop=True)
            mr = singles.tile([1, 2 * K], F32, tag=f"mr{l}")
            nc.vector.tensor_scalar(mr[:], st_ps[:], 1.0 / C, None, ALU.mult)
            # var = E[x^2] - mu^2 ; rs = 1/sqrt(var+eps) via newton (vector)
            vs8 = singles.tile([1, 2 * K], F32, tag=f"vs8{l}")
            nc.vector.tensor_tensor(vs8[:, :K], mr[:, :K], mr[:, :K], ALU.mult)
            nc.vector.tensor_tensor(vs8[:, :K], mr[:, K:], vs8[:, :K],
                                    ALU.subtract)
            nc.vector.tensor_scalar(vs8[:, :K], vs8[:, :K], 1e-6, None,
                                    ALU.add)
            rsqrt_newton(vs8[:, :K], mr[:, K:], vs8[:, K:], 1, K, iters=1)
            # broadcast [mu | rs] down partitions
            bc_ps = psD.tile([C, 2 * K], F32, tag="pd")
            nc.tensor.matmul(bc_ps[:], ones_row[:], mr[:], start=True,
                             stop=True)
            fea2 = singles.tile([C, K], F32, tag=f"fea{l+1}")
            nc.vector.tensor_tensor(fea2[:], stin[:, :K], bc_ps[:, :K],
                                    ALU.subtract)
            nc.vector.tensor_tensor(fea2[:], fea2[:], bc_ps[:, K:], ALU.mult)
            nc.vector.tensor_scalar(fea2[:], fea2[:], lnw_c, lnb_c,
                                    ALU.mult, ALU.add)
            fea = fea2[:]
            # x^2 channel sums, interleaved into scalar's idle gaps
            for ci in range(l * 5, min(l * 5 + (5 if l < 2 else 6),
                                       N // LOADCH)):
                sqscr = scr2.tile([C, LOADCH], BF16, tag="sqscr")
                nc.scalar.activation(sqscr[:],
                                     xb[:, ci * LOADCH:(ci + 1) * LOADCH],
                                     AF.Square,
                                     accum_out=xsq_part[:, ci:ci + 1])

        # ---------------- closed-form instance/batch-norm stats -----------
        ftp = psA.tile([C, C], F32, tag="pa")
        nc.tensor.transpose(ftp[:K, :], fea, ident[:])
        featb = singles.tile([K, C], BF16)
        nc.vector.tensor_copy(featb[:], ftp[:K, :])
        featr = singles.tile([K, C], F32)
        nc.vector.tensor_copy(featr[:], featb[:])
        fsq = singles.tile([K, C], F32)
        nc.vector.tensor_tensor(fsq[:], featr[:], featr[:], ALU.mult)
        prod = singles.tile([K, C], F32)
        nc.vector.tensor_tensor(prod[:], featr[:], pooledT, ALU.mult)
        r1_ps = psD.tile([C, 1], F32, tag="pd")
        nc.tensor.matmul(r1_ps[:], featr[:], nums_c[:], start=True, stop=True)
        r2_ps = psD.tile([C, 1], F32, tag="pd")
        nc.tensor.matmul(r2_ps[:], fsq[:], nums_c[:], start=True, stop=True)
        r3_ps = psA.tile([C, 1], F32, tag="pa")
        nc.tensor.matmul(r3_ps[:], prod[:], ones8[:], start=True, stop=True)

        xsq_col = singles.tile([C, 1], F32)
        nc.vector.reduce_sum(xsq_col[:], xsq_part[:], axis=AX.X)
        stats = singles.tile([C, 8], F32)
        mu = stats[:, 0:1]
        var = stats[:, 1:2]
        rsb = stats[:, 2:3]
        s_col = stats[:, 3:4]
        b_col = stats[:, 4:5]
        tmp = stats[:, 5:6]
        nc.vector.tensor_tensor(tmp[:], sumx_col, r1_ps[:], ALU.add)
        nc.vector.tensor_scalar(mu[:], tmp[:], 1.0 / N, None, ALU.mult)
        nc.vector.tensor_scalar(tmp[:], r3_ps[:], 2.0, None, ALU.mult)
        nc.vector.tensor_tensor(tmp[:], tmp[:], r2_ps[:], ALU.add)
        nc.vector.tensor_tensor(tmp[:], tmp[:], xsq_col[:], ALU.add)
        nc.vector.tensor_scalar(tmp[:], tmp[:], 1.0 / N, None, ALU.mult)
        nc.vector.tensor_tensor(var[:], mu[:], mu[:], ALU.mult)
        nc.vector.tensor_tensor(var[:], tmp[:], var[:], ALU.subtract)
        # rs_i = 1/sqrt(var+1e-5); vb = var/(var+1e-5); rs_b = 1/sqrt(vb+1e-5)
        rsi = stats[:, 6:7]
        inv = stats[:, 7:8]
        scr2c = singles.tile([C, 2], F32, tag="nsc")
        nc.vector.tensor_scalar(inv[:], var[:], 1e-5, None, ALU.add)
        rsqrt_newton(inv[:], rsi[:], scr2c[:, 0:1], C, 1)
        nc.vector.reciprocal(inv[:], inv[:])
        nc.vector.tensor_tensor(rsb[:], var[:], inv[:], ALU.mult)
        nc.vector.tensor_scalar(rsb[:], rsb[:], 1e-5, None, ALU.add)
        rsqrt_newton(rsb[:], scr2c[:, 1:2], scr2c[:, 0:1], C, 1)
        nc.vector.tensor_copy(rsb[:], scr2c[:, 1:2])
        # dummy gelu: pull the gelu-table load off the critical path
        nc.scalar.activation(dummy[:], dummy[:], AF.Gelu)
        nc.vector.tensor_tensor(s_col[:], rsi[:], rsb[:], ALU.mult)
        nc.vector.tensor_tensor(s_col[:], s_col[:], bnw_c, ALU.mult)
        nc.vector.tensor_tensor(b_col[:], mu[:], s_col[:], ALU.mult)
        nc.vector.tensor_tensor(b_col[:], bnb_c, b_col[:], ALU.subtract)

        psD_cm.__exit__(None, None, None)

        # ---------------- pass R + gates + pass F ----------------
        g = singles.tile([C, N], BF16)
        gsum_part = singles.tile([C, N // CH], F32)
        with tc.tile_pool(name="psC", bufs=3, space="PSUM") as psC:
            for r in range(N // CH):
                off = r * CH
                sc_ps = psC.tile([C, CH], F32, tag="pc")
                for h in range(CH // 512):
                    o2 = off + h * 512
                    sl = sc_ps[:, h * 512:(h + 1) * 512]
                    nc.tensor.matmul(sl, identb[:], xb[:, o2:o2 + 512],
                                     start=True, stop=False)
                    nc.tensor.matmul(sl, featb[:], Mrow[:, o2:o2 + 512],
                                     start=False, stop=True)
                nc.scalar.activation(g[:, off:off + CH], sc_ps[:], AF.Gelu,
                                     bias=b_col, scale=s_col,
                                     accum_out=gsum_part[:, r:r + 1])

            # SE gates
            gsum_col = singles.tile([C, 1], F32)
            nc.vector.reduce_sum(gsum_col[:], gsum_part[:], axis=AX.X)
            sq_ps = psA.tile([C, 1], F32, tag="pa")
            nc.tensor.matmul(sq_ps[:], convwT[:], gsum_col[:], start=True,
                             stop=True)
            sq = singles.tile([C, 1], F32)
            nc.vector.tensor_scalar(sq[:], sq_ps[:], 1.0 / N, convb_c,
                                    ALU.mult, ALU.add)
            f1_ps = psA.tile([H, 1], F32, tag="pa")
            nc.tensor.matmul(f1_ps[:], fc1wT[:], sq[:], start=True, stop=True)
            f1 = singles.tile([H, 1], F32)
            nc.scalar.activation(f1[:], f1_ps[:], AF.Gelu, bias=fc1b_c)
            f2_ps = psA.tile([C, 1], F32, tag="pa")
            nc.tensor.matmul(f2_ps[:], fc2wT[:], f1[:], start=True, stop=True)
            # sigmoid(z) = 0.5*tanh(z/2) + 0.5 (tanh is in the gelu table)
            f2 = singles.tile([C, 1], F32)
            nc.scalar.activation(f2[:], f2_ps[:], AF.Tanh, scale=0.5,
                                 bias=halffc2b_c)
            nc.vector.tensor_scalar(f2[:], f2[:], 0.5, 0.5, ALU.mult, ALU.add)
            fb = singles.tile([C, 1], F32)     # f2 * conv0_b
            nc.vector.tensor_tensor(fb[:], f2[:], convb_c, ALU.mult)
            # W''[c, o] = convwT[c, o] * f2[o]  via  conv0_w @ diag(f2)
            diag = singles.tile([C, C], F32)
            nc.vector.tensor_scalar(diag[:], ident[:], f2[:], None, ALU.mult)
            wpp_ps = psA.tile([C, C], F32, tag="pa")
            nc.tensor.matmul(wpp_ps[:], wsb["conv0_w"][:], diag[:], start=True,
                             stop=True)
            wpp = singles.tile([C, C], BF16)
            nc.vector.tensor_copy(wpp[:], wpp_ps[:])

            # pass F: conv + bias-row matmul -> copy -> store
            for r in range(N // CH):
                off = r * CH
                cv_ps = psC.tile([C, CH], F32, tag="pc")
                for h in range(CH // 512):
                    o2 = off + h * 512
                    nc.tensor.matmul(cv_ps[:, h * 512:(h + 1) * 512], wpp[:],
                                     g[:, o2:o2 + 512], start=True, stop=True)
                ot = och.tile([C, CH], BF16, tag="ot")
                if r % 2 == 0:
                    nc.vector.tensor_scalar(ot[:], cv_ps[:], fb[:], None,
                                            ALU.add)
                else:
                    nc.scalar.activation(ot[:], cv_ps[:], AF.Identity,
                                         bias=fb[:])
                nc.sync.dma_start(out_d.ap()[:, off:off + CH], ot[:])


_NC_CACHE = {}


def _get_nc():
    if "nc" not in _NC_CACHE:
        _NC_CACHE["nc"] = build_nc()
    return _NC_CACHE["nc"]


def kernel(**inputs):
    x = np.ascontiguousarray(np.asarray(inputs["x"], dtype=np.float32))
    logits = np.ascontiguousarray(np.asarray(inputs["logits"],
                                             dtype=np.float32))
    assert x.shape == (B, C, N, 1) and logits.shape == (B, N)
    ident = np.eye(C, dtype=np.float32)
    shared = {"ident": ident}
    for nm in ("Wq1", "Wk1", "Wv1", "Wq2", "Wk2", "Wv2", "Wq3", "Wk3", "Wv3",
               "conv0_w", "fc1_w", "fc2_w", "ln_w", "ln_b", "bn_w", "bn_b",
               "conv0_b", "fc1_b", "fc2_b"):
        shared[nm] = np.ascontiguousarray(np.asarray(inputs[nm],
                                                     dtype=np.float32))
    in_maps = []
    for i in range(NCORES):
        m = dict(shared)
        m["x"] = np.ascontiguousarray(x[i, :, :, 0])
        m["logits"] = np.ascontiguousarray(logits[i])
        in_maps.append(m)

    nc = _get_nc()
    res = run_bass_kernel_spmd(nc, in_maps, list(range(NCORES))).results
    out = np.stack([res[i]["out"] for i in range(NCORES)], axis=0)
    return out[..., None].astype(np.float32)
